# revision 13
# baseline (speedup 1.0000x reference)
"""AttentionRGCN layer on 8 Trainium2 NeuronCores (Bass/Tile).

Math notes (vs the jax reference):
  - alpha = exp(leaky_relu(score)) normalized over axis=1 of an [E,1] tensor
    is exactly 1.0 for every edge (x/x with finite positive x), so the whole
    attention branch (W3, a, score) drops out of the computation.
  - matmul commutes with segment_sum:
        segsum((h[src]+rel[etype]) @ Wn, dst) = segsum(h[src]+rel[etype], dst) @ Wn
    and segsum(rel[etype], dst) = C @ rel where C[n, r] counts (dst=n, etype=r)
    edges.  C is pure index data, so it is built host-side during sharding.
  So per node:  out = relu((S + C@rel) @ Wn) + (indeg>0 ? h@Lw : h@Ew)
  with S = segsum(h[src], dst).

Sharding: destination-node sharding.  Each core owns NBLK blocks of 128 node
slots; a host-side 2D bin-packing permutes nodes into blocks so that each
block's edge count per src-half (h row index split at SPLIT for the int16
gather index limit) fits a uniform per-block group budget.  Edge h[src] rows
are fetched with gpsimd.dma_gather; the scatter-add into the owning block is
a TensorEngine matmul against per-group one-hot matrices built on DVE with
an is_equal compare against an iota row.  Everything downstream runs in the
feature-on-partition ("transposed") orientation so no activation transposes
are needed except one PE transpose of the core's own h slab.
"""

import sys

for _p in ("/opt/trn_rl_repo",):
    if _p not in sys.path:
        sys.path.append(_p)

import numpy as np
import ml_dtypes

from concourse import bacc, bass, mybir, tile
from concourse import library_config
from concourse.masks import make_identity
from concourse.bass_utils import run_bass_kernel_spmd

P = 128
D = 256  # feature dim (fixed by problem)
NCORES = 8
BF16 = mybir.dt.bfloat16
F32 = mybir.dt.float32
I16 = mybir.dt.int16
np_bf16 = ml_dtypes.bfloat16


# ----------------------------------------------------------------------------
# Host-side sharding / packing (index work only -- no float math on h)
# ----------------------------------------------------------------------------

def _pack_nodes(deg_lo, deg_hi, n_nodes, ncores, nblk, tail_reserve):
    """Assign nodes to ncores*nblk bins of <=128 nodes, balancing per-bin
    (lo, hi) edge sums.  Zero-degree nodes are NOT passed here.
    tail_reserve[b] = slots reserved in bin b (for zero-deg + dummy tail).
    Returns bin_of_node-ish arrays: lists of node ids per bin."""
    nbins = ncores * nblk
    deg = deg_lo + deg_hi
    order = np.argsort(-deg, kind="stable")
    bins_lo = np.zeros(nbins, np.int64)
    bins_hi = np.zeros(nbins, np.int64)
    bins_cnt = np.zeros(nbins, np.int64)
    cap_cnt = np.full(nbins, P, np.int64) - tail_reserve
    members = [[] for _ in range(nbins)]
    dl = deg_lo[order]
    dh = deg_hi[order]
    for i, n in enumerate(order):
        lo = bins_lo + dl[i]
        hi = bins_hi + dh[i]
        score = np.maximum(lo, hi).astype(np.float64)
        score[bins_cnt >= cap_cnt] = np.inf
        b = int(np.argmin(score))
        members[b].append(int(n))
        bins_lo[b] += dl[i]
        bins_hi[b] += dh[i]
        bins_cnt[b] += 1
    return members, bins_lo, bins_hi


def host_prep(h, rel_emb, weight_neighbor, loop_weight, evolve_loop_weight,
              src, dst, etype, ncores=NCORES):
    """Build all per-core device input arrays + the structural metadata the
    graph builder needs.  Only integer/index work plus array slicing here."""
    n_nodes, d = h.shape
    assert d == D
    n_rel = rel_emb.shape[0]
    e = src.shape[0]
    src = np.asarray(src).astype(np.int64)
    dst = np.asarray(dst).astype(np.int64)
    etype = np.asarray(etype).astype(np.int64)

    split = min((n_nodes + 1) // 2, 32000)
    nblk = -(-n_nodes // (ncores * P))  # ceil
    # keep (nblk - ls) % 4 == 0 so full 512-wide output tiles align; ls >= 1
    ls = 1 + (nblk - 1) % 4
    slots = nblk * P

    e_half = (src >= split).astype(np.int64)
    deg_lo = np.bincount(dst[e_half == 0], minlength=n_nodes)
    deg_hi = np.bincount(dst[e_half == 1], minlength=n_nodes)
    deg = deg_lo + deg_hi

    zz = np.where(deg == 0)[0]
    nz = np.where(deg > 0)[0]
    n_dummy = ncores * slots - n_nodes
    # distribute zero-deg nodes round-robin over cores' tails
    z_per_core = [zz[c::ncores] for c in range(ncores)]
    tail_len = np.array([len(z) for z in z_per_core], np.int64)
    # dummies: n_dummy total, split as evenly as possible
    dummy_per_core = np.full(ncores, n_dummy // ncores, np.int64)
    dummy_per_core[: n_dummy % ncores] += 1
    tail_total = tail_len + dummy_per_core
    assert np.all(tail_total <= ls * P), (
        f"zero-degree tail {tail_total} exceeds select region {ls * P}"
    )

    nbins = ncores * nblk
    tail_reserve = np.zeros(nbins, np.int64)
    # reserve at the END of each core's bin range
    for c in range(ncores):
        rem = int(tail_total[c])
        b = nblk - 1
        while rem > 0:
            take = min(rem, P)
            tail_reserve[c * nblk + b] = take
            rem -= take
            b -= 1

    members, _, _ = _pack_nodes(
        deg_lo[nz], deg_hi[nz], len(nz), ncores, nblk, tail_reserve
    )
    members = [[nz[i] for i in m] for m in members]

    # append tails: zero-deg nodes then dummies (node id -1)
    for c in range(ncores):
        tail_nodes = list(z_per_core[c]) + [-1] * int(dummy_per_core[c])
        b = nblk - 1
        while tail_nodes:
            room = P - len(members[c * nblk + b])
            members[c * nblk + b].extend(tail_nodes[:room])
            tail_nodes = tail_nodes[room:]
            b -= 1

    node_of_slot = np.full((ncores, slots), -1, np.int64)
    for c in range(ncores):
        for b in range(nblk):
            m = members[c * nblk + b]
            assert len(m) <= P
            node_of_slot[c, b * P : b * P + len(m)] = m

    slot_of_node = np.full(n_nodes, -1, np.int64)
    flat = node_of_slot.reshape(-1)
    valid = flat >= 0
    slot_of_node[flat[valid]] = np.arange(ncores * slots)[valid]
    assert np.all(slot_of_node >= 0)

    gslot = slot_of_node[dst]           # global slot of each edge's dst
    e_core = gslot // slots
    e_local = gslot % slots
    e_blk = e_local // P
    e_lane = e_local % P

    # per-(core, blk, half) counts -> uniform group caps
    cnt = np.zeros((ncores, nblk, 2), np.int64)
    np.add.at(cnt, (e_core, e_blk, e_half), 1)
    cap_lo = max(1, int(-(-cnt[:, :, 0].max() // P)))
    cap_hi = max(1, int(-(-cnt[:, :, 1].max() // P)))

    arena_slots = (nblk * cap_lo * P, nblk * cap_hi * P)
    gidx = [np.zeros((ncores, arena_slots[a]), np.int64) for a in range(2)]
    dloc = [np.full((ncores, arena_slots[a]), 200.0, np.float32) for a in range(2)]

    order = np.lexsort((e_half, e_blk, e_core))
    sc, sb, sh = e_core[order], e_blk[order], e_half[order]
    slane = e_lane[order]
    ssrc = src[order]
    # segment starts over (core, blk, half)
    key = (sc * nblk + sb) * 2 + sh
    seg_starts = np.searchsorted(key, np.arange(ncores * nblk * 2))
    seg_ends = np.append(seg_starts[1:], e)
    cap_arr = (cap_lo * P, cap_hi * P)
    for c in range(ncores):
        for b in range(nblk):
            for a in range(2):
                k = (c * nblk + b) * 2 + a
                s0, s1 = seg_starts[k], seg_ends[k]
                cnt_e = s1 - s0
                assert cnt_e <= cap_arr[a]
                base = b * cap_arr[a]
                gidx[a][c, base : base + cnt_e] = ssrc[s0:s1] - (split if a else 0)
                dloc[a][c, base : base + cnt_e] = slane[s0:s1]

    # C^T: counts of (slot, etype) per core -> [n_rel, slots]
    ct = np.zeros((ncores, n_rel, slots), np.float32)
    np.add.at(ct, (e_core, etype, e_local), 1.0)

    # own h rows (dummy slots -> zeros)
    h_own = np.zeros((ncores, slots, D), np.float32)
    for c in range(ncores):
        ns = node_of_slot[c]
        v = ns >= 0
        h_own[c][v] = h[ns[v]]

    # select masks for the tail region (ls blocks): 1 -> Lw, 0 -> Ew
    z_needed = len(zz) > 0
    selm = np.ones((ncores, ls * P), np.float32)
    for c in range(ncores):
        ns = node_of_slot[c, (nblk - ls) * P :]
        selm[c] = ((ns >= 0) & (deg[np.maximum(ns, 0)] > 0)).astype(np.float32)

    # wrap gather indices per gather-call chunk of GPC groups (<=1024 idxs,
    # the HW per-dma_gather limit at 1KB rows) -> [16, n/16] tiled to 128 rows
    gpc = 8

    def wrap_idx(arr_a, cap):
        out = []
        ng = nblk * cap
        for c in range(ncores):
            rows = []
            for g0 in range(0, ng, gpc):
                gl = min(gpc, ng - g0)
                seg = arr_a[c, g0 * P : (g0 + gl) * P]
                w = seg.reshape(-1, 16).T  # [16, n/16]
                rows.append(np.tile(w, (8, 1)))
            out.append(np.concatenate(rows, axis=1))
        return np.stack(out).astype(np.int16)

    gidx_w = [wrap_idx(gidx[a], (cap_lo, cap_hi)[a]) for a in range(2)]
    dloc_w = [
        np.stack([dloc[a][c].reshape(-1, P).T for c in range(ncores)]).astype(
            np.float32)
        for a in range(2)
    ]

    iota = np.tile(np.arange(P, dtype=np.float32)[None, :], (P, 1))

    in_maps = []
    for c in range(ncores):
        in_maps.append({
            "h_lo": np.ascontiguousarray(h[:split]).astype(np.float32),
            "h_hi": np.ascontiguousarray(h[split:]).astype(np.float32),
            "h_own": h_own[c],
            "gidx_lo": gidx_w[0][c],
            "gidx_hi": gidx_w[1][c],
            "dloc_lo": dloc_w[0][c],
            "dloc_hi": dloc_w[1][c],
            "ct": ct[c].astype(np_bf16),
            "rel_emb": rel_emb.astype(np.float32),
            "w_n": weight_neighbor.astype(np.float32),
            "w_l": loop_weight.astype(np.float32),
            "w_e": evolve_loop_weight.astype(np.float32),
            "iota": iota,
            "selm": np.tile(selm[c][None, :], (P, 1)).astype(np_bf16),
        })

    meta = dict(
        n_nodes=n_nodes, n_rel=n_rel, split=split, nblk=nblk, ls=ls,
        slots=slots, cap_lo=cap_lo, cap_hi=cap_hi, gpc=gpc,
        z_needed=bool(z_needed), node_of_slot=node_of_slot,
    )
    return in_maps, meta


# ----------------------------------------------------------------------------
# Device graph
# ----------------------------------------------------------------------------

def build_graph(meta, ncores=NCORES):
    n_nodes = meta["n_nodes"]
    n_rel = meta["n_rel"]
    split = meta["split"]
    nblk = meta["nblk"]
    ls = meta["ls"]
    slots = meta["slots"]
    cap_lo = meta["cap_lo"]
    cap_hi = meta["cap_hi"]
    gpc = meta["gpc"]
    z_needed = meta["z_needed"]
    caps = (cap_lo, cap_hi)

    nc = bacc.Bacc("TRN2", target_bir_lowering=False, debug=False,
                   num_devices=ncores)

    dt = nc.dram_tensor
    h_lo = dt("h_lo", [split, D], F32, kind="ExternalInput")
    h_hi = dt("h_hi", [n_nodes - split, D], F32, kind="ExternalInput")
    h_own = dt("h_own", [slots, D], F32, kind="ExternalInput")
    gidx = [dt("gidx_lo", [P, nblk * cap_lo * 8], I16, kind="ExternalInput"),
            dt("gidx_hi", [P, nblk * cap_hi * 8], I16, kind="ExternalInput")]
    dloc = [dt("dloc_lo", [P, nblk * cap_lo], F32, kind="ExternalInput"),
            dt("dloc_hi", [P, nblk * cap_hi], F32, kind="ExternalInput")]
    ct = dt("ct", [n_rel, slots], BF16, kind="ExternalInput")
    rel = dt("rel_emb", [n_rel, D], F32, kind="ExternalInput")
    w_n = dt("w_n", [D, D], F32, kind="ExternalInput")
    w_l = dt("w_l", [D, D], F32, kind="ExternalInput")
    w_e = dt("w_e", [D, D], F32, kind="ExternalInput")
    iota = dt("iota", [P, P], F32, kind="ExternalInput")
    selm = dt("selm", [P, ls * P], BF16, kind="ExternalInput")
    out = dt("out", [2, P, slots], F32, kind="ExternalOutput")

    h_src = [h_lo, h_hi]
    rel_k = [(k, min(P, n_rel - k)) for k in range(0, n_rel, P)]

    with tile.TileContext(nc) as tc:
        with (
            tc.tile_pool(name="persist", bufs=1) as pp,
            tc.tile_pool(name="gather", bufs=3) as gp,
            tc.tile_pool(name="gbf", bufs=3) as gbp,
            tc.tile_pool(name="oh", bufs=8) as ohp,
            tc.tile_pool(name="wrk", bufs=3) as wp,
            tc.tile_pool(name="epi", bufs=4) as ep,
            tc.tile_pool(name="psum", bufs=8, space="PSUM") as psp,
        ):
            # ---- persistent small tensors ------------------------------------
            nc.gpsimd.load_library(library_config.mlp)
            ident = pp.tile([P, P], F32, tag="ident")
            make_identity(nc, ident[:])
            iota_t = pp.tile([P, P], F32, tag="iota")
            nc.sync.dma_start(iota_t[:], iota.ap())
            dloc_t = [pp.tile([P, nblk * caps[a]], F32, tag=f"dloc{a}",
                              name=f"dloc_t{a}") for a in range(2)]
            for a in range(2):
                nc.sync.dma_start(dloc_t[a][:], dloc[a].ap())
            gidx_t = [pp.tile([P, nblk * caps[a] * 8], I16, tag=f"gidx{a}",
                              name=f"gidx_t{a}") for a in range(2)]
            for a in range(2):
                nc.sync.dma_start(gidx_t[a][:], gidx[a].ap())
            selm_t = pp.tile([P, ls * P], BF16, tag="selm")
            nc.sync.dma_start(selm_t[:], selm.ap())
            ct_t = [pp.tile([kn, slots], BF16, tag=f"ct{k}", name=f"ct_t{k}")
                    for k, kn in rel_k]
            for i, (k, kn) in enumerate(rel_k):
                nc.sync.dma_start(ct_t[i][:], ct.ap()[k:k + kn, :])

            # weights: load f32, cast to bf16 [K=D rows on partitions]
            def load_w_bf(w, tag):
                tiles = []
                for kc in range(D // P):
                    wf = wp.tile([P, D], F32, tag="wf32")
                    nc.sync.dma_start(wf[:], w.ap()[kc * P:(kc + 1) * P, :])
                    wb = pp.tile([P, D], BF16, tag=f"{tag}{kc}")
                    nc.vector.tensor_copy(wb[:], wf[:])
                    tiles.append(wb)
                return tiles

            w_n_t = load_w_bf(w_n, "wn")
            w_l_t = load_w_bf(w_l, "wl")
            w_e_t = load_w_bf(w_e, "we") if z_needed else None

            # rel_w = rel_emb @ Wn, computed on-device:
            #   transpose rel chunks to relT[fh] = rel^T (feat-half on parts),
            #   then rel_w chunk = (relT chunk-slice)^T-matmul against Wn.
            relT = [pp.tile([P, n_rel], BF16, tag=f"relT{kc}",
                            name=f"relT{kc}") for kc in range(2)]
            for ki, (k, kn) in enumerate(rel_k):
                rf = wp.tile([kn, D], F32, tag="relf32")
                nc.sync.dma_start(rf[:], rel.ap()[k:k + kn, :])
                for kc in range(2):
                    ps = psp.tile([P, P], F32, tag="ps")
                    nc.tensor.transpose(ps[:, :kn], rf[:, kc * P:(kc + 1) * P],
                                        ident[:kn, :kn])
                    nc.vector.tensor_copy(relT[kc][:, k:k + kn], ps[:, :kn])
            relw_t = []
            for ki, (k, kn) in enumerate(rel_k):
                ps = psp.tile([P, D], F32, tag="ps")
                for kc in range(2):
                    nc.tensor.matmul(ps[:kn, :], relT[kc][:, k:k + kn],
                                     w_n_t[kc][:], start=(kc == 0),
                                     stop=(kc == 1))
                rw = pp.tile([kn, D], BF16, tag=f"relw{k}", name=f"relw{k}")
                nc.vector.tensor_copy(rw[:], ps[:kn, :])
                relw_t.append(rw)

            # ---- S^T accumulation: gather + one-hot matmuls ------------------
            import os as _os
            _skip = set(_os.environ.get("KSKIP", "").split(","))
            st = pp.tile([P, 2, slots], BF16, tag="st")  # [feat-half][f, slot]
            if "units" in _skip:
                nc.vector.memset(st[:], 0.0)

            # gather call k of arena a covers groups [k*gpc, k*gpc+gl)
            g_tiles = [{}, {}]

            def ensure_call(a, call):
                if call in g_tiles[a]:
                    return
                cap = caps[a]
                ng = nblk * cap
                g0 = call * gpc
                gl = min(gpc, ng - g0)
                n_idx = gl * P
                gf = gp.tile([P, gpc, D], F32, tag=f"g{a}",
                             name=f"gf_{a}_{call}")
                nc.gpsimd.dma_gather(
                    gf[:, :gl, :], h_src[a].ap(),
                    gidx_t[a][:, g0 * 8:(g0 + gl) * 8],
                    n_idx, n_idx, D,
                )
                gb = gbp.tile([P, gpc, D], BF16, tag=f"gb{a}",
                              name=f"gb_{a}_{call}")
                nc.vector.tensor_copy(gb[:, :gl, :], gf[:, :gl, :])
                g_tiles[a][call] = gb

            for b in ([] if "units" in _skip else range(nblk)):
                for a in range(2):
                    for g in range(caps[a]):
                        ensure_call(a, (b * caps[a] + g) // gpc)
                ohs = [[], []]
                for a in range(2):
                    for g in range(caps[a]):
                        oh = ohp.tile([P, P], BF16, tag="oh")
                        nc.vector.tensor_scalar(
                            oh[:], iota_t[:],
                            dloc_t[a][:, b * caps[a] + g: b * caps[a] + g + 1],
                            None, mybir.AluOpType.is_equal,
                        )
                        ohs[a].append(oh)
                for fh in range(2):
                    ps = psp.tile([P, P], F32, tag="ps")
                    tot = caps[0] + caps[1]
                    i = 0
                    for a in range(2):
                        for g in range(caps[a]):
                            gg = b * caps[a] + g
                            gb = g_tiles[a][gg // gpc]
                            nc.tensor.matmul(
                                ps[:],
                                gb[:, gg % gpc, fh * P:(fh + 1) * P],
                                ohs[a][g][:],
                                start=(i == 0), stop=(i == tot - 1),
                            )
                            i += 1
                    nc.vector.tensor_copy(st[:, fh, b * P:(b + 1) * P], ps[:])

            # ---- h_own transpose: [slots, D] f32 -> h_ownT bf16 --------------
            ht = pp.tile([P, 2, slots], BF16, tag="ht")
            if "ht" in _skip:
                nc.vector.memset(ht[:], 0.0)
            for mb in ([] if "ht" in _skip else range(nblk)):
                hf = wp.tile([P, D], F32, tag="hf32")
                nc.sync.dma_start(hf[:], h_own.ap()[mb * P:(mb + 1) * P, :])
                for fh in range(2):
                    ps = psp.tile([P, P], F32, tag="ps")
                    nc.tensor.transpose(ps[:], hf[:, fh * P:(fh + 1) * P], ident[:])
                    nc.vector.tensor_copy(ht[:, fh, mb * P:(mb + 1) * P], ps[:])

            # ---- epilogue ----------------------------------------------------
            # out^T[fh] = relu(Wn^T@S^T + rel^T@C^T) + sel(Lw^T@h^T, Ew^T@h^T)
            ntiles = [(t * 512, 512) for t in range((nblk - ls) * P // 512)]
            ntiles += [((nblk - ls) * P + t * P, P) for t in range(ls)]
            sel_from = (nblk - ls) * P

            for fh in range(2):
                for n0, nw in ntiles:
                    in_sel = z_needed and n0 >= sel_from
                    agg = psp.tile([P, nw], F32, tag="ps")
                    nmm = 2 + len(rel_k)
                    i = 0
                    for kc in range(2):
                        nc.tensor.matmul(
                            agg[:], w_n_t[kc][:, fh * P:(fh + 1) * P],
                            st[:, kc, n0:n0 + nw],
                            start=(i == 0), stop=(i == nmm - 1))
                        i += 1
                    for ki, (k, kn) in enumerate(rel_k):
                        nc.tensor.matmul(
                            agg[:], relw_t[ki][:, fh * P:(fh + 1) * P],
                            ct_t[ki][:, n0:n0 + nw],
                            start=(i == 0), stop=(i == nmm - 1))
                        i += 1
                    lp = psp.tile([P, nw], F32, tag="ps")
                    for kc in range(2):
                        nc.tensor.matmul(
                            lp[:], w_l_t[kc][:, fh * P:(fh + 1) * P],
                            ht[:, kc, n0:n0 + nw],
                            start=(kc == 0), stop=(kc == 1))
                    ot = ep.tile([P, nw], F32, tag="ot")
                    nc.scalar.activation(ot[:], agg[:],
                                         mybir.ActivationFunctionType.Relu)
                    if not in_sel:
                        nc.vector.tensor_add(ot[:], ot[:], lp[:])
                    else:
                        ep_ps = psp.tile([P, nw], F32, tag="ps")
                        for kc in range(2):
                            nc.tensor.matmul(
                                ep_ps[:], w_e_t[kc][:, fh * P:(fh + 1) * P],
                                ht[:, kc, n0:n0 + nw],
                                start=(kc == 0), stop=(kc == 1))
                        lsb = ep.tile([P, nw], F32, tag="lsb")
                        m0 = n0 - sel_from
                        nc.vector.tensor_tensor(
                            lsb[:], lp[:], selm_t[:, m0:m0 + nw],
                            op=mybir.AluOpType.mult)
                        nc.vector.tensor_add(ot[:], ot[:], lsb[:])
                        # (1 - m) * E = E - m*E
                        me = ep.tile([P, nw], F32, tag="me")
                        nc.vector.tensor_tensor(
                            me[:], ep_ps[:], selm_t[:, m0:m0 + nw],
                            op=mybir.AluOpType.mult)
                        nc.vector.tensor_tensor(
                            me[:], ep_ps[:], me[:],
                            op=mybir.AluOpType.subtract)
                        nc.vector.tensor_add(ot[:], ot[:], me[:])
                    nc.sync.dma_start(out.ap()[fh, :, n0:n0 + nw], ot[:])

    nc.compile()
    return nc


# ----------------------------------------------------------------------------
# Entry point
# ----------------------------------------------------------------------------

_CACHE = {}


def _run(inputs, ncores=NCORES, sim=False, trace=False):
    h = np.asarray(inputs["h"], np.float32)
    rel_emb = np.asarray(inputs["rel_emb"], np.float32)
    w_n = np.asarray(inputs["weight_neighbor"], np.float32)
    w_l = np.asarray(inputs["loop_weight"], np.float32)
    w_e = np.asarray(inputs["evolve_loop_weight"], np.float32)
    src = np.asarray(inputs["src"])
    dst = np.asarray(inputs["dst"])
    etype = np.asarray(inputs["etype"])

    in_maps, meta = host_prep(h, rel_emb, w_n, w_l, w_e, src, dst, etype,
                              ncores=ncores)

    key = (meta["n_nodes"], meta["n_rel"], meta["split"], meta["nblk"],
           meta["cap_lo"], meta["cap_hi"], meta["ls"], meta["z_needed"], ncores)
    if key not in _CACHE:
        _CACHE[key] = build_graph(meta, ncores=ncores)
    nc = _CACHE[key]

    res = run_bass_kernel_spmd(nc, in_maps, core_ids=list(range(ncores)),
                               trace=trace)

    n_nodes = meta["n_nodes"]
    slots = meta["slots"]
    node_of_slot = meta["node_of_slot"]
    out_full = np.zeros((n_nodes, D), np.float32)
    for c in range(ncores):
        oc = np.asarray(res.results[c]["out"])  # [2, P, slots]
        oc = oc.reshape(2 * P, slots)  # [D(feature), slots]
        ns = node_of_slot[c]
        v = ns >= 0
        out_full[ns[v]] = oc[:, v].T
    return out_full, res, meta


def kernel(**inputs) -> np.ndarray:
    out, _, _ = _run(inputs)
    return out.astype(np.float32)


if __name__ == "__main__":
    # tiny smoke test with a synthetic small graph
    rng = np.random.default_rng(0)
    N_, E_, R_ = 2048, 16384, 16
    inputs = dict(
        h=rng.standard_normal((N_, D), dtype=np.float32),
        rel_emb=(rng.standard_normal((R_, D)) * 0.1).astype(np.float32),
        weight_neighbor=rng.standard_normal((D, D), dtype=np.float32) * 0.05,
        a=np.zeros((3 * D, 1), np.float32),
        W3=np.zeros((3 * D, 3 * D), np.float32),
        loop_weight=rng.standard_normal((D, D), dtype=np.float32) * 0.05,
        evolve_loop_weight=rng.standard_normal((D, D), dtype=np.float32) * 0.05,
        src=rng.integers(0, N_, E_),
        dst=rng.integers(0, N_, E_),
        etype=rng.integers(0, R_, E_),
    )
    out, res, meta = _run({k: v for k, v in inputs.items()
                           if k not in ("a", "W3")})
    # numpy reference
    S = np.zeros((N_, D), np.float32)
    np.add.at(S, inputs["dst"], inputs["h"][inputs["src"]]
              + inputs["rel_emb"][inputs["etype"]])
    indeg = np.bincount(inputs["dst"], minlength=N_)
    loopm = np.where((indeg > 0)[:, None],
                     inputs["h"] @ inputs["loop_weight"],
                     inputs["h"] @ inputs["evolve_loop_weight"])
    ref = np.maximum(S @ inputs["weight_neighbor"], 0) + loopm
    err = np.abs(out - ref).max() / (np.abs(ref).max() + 1e-9)
    print("small-graph rel err:", err)


# revision 32
# speedup vs baseline: 2.0061x; 2.0061x over previous
"""AttentionRGCN layer on 8 Trainium2 NeuronCores (Bass/Tile).

Math notes (vs the jax reference):
  - alpha = exp(leaky_relu(score)) normalized over axis=1 of an [E,1] tensor
    is exactly 1.0 for every edge (x/x with finite positive x), so the whole
    attention branch (W3, a, score) drops out of the computation.
  - matmul commutes with segment_sum:
        segsum((h[src]+rel[etype]) @ Wn, dst) = segsum(h[src]+rel[etype], dst) @ Wn
    and segsum(rel[etype], dst) = C @ rel where C[n, r] counts (dst=n, etype=r)
    edges.  C is pure index data, so it is built host-side during sharding.
  So per node:  out = relu((S + C@rel) @ Wn) + (indeg>0 ? h@Lw : h@Ew)
  with S = segsum(h[src], dst).

Sharding: destination-node sharding.  Each core owns NBLK blocks of 128 node
slots; a host-side 2D bin-packing permutes nodes into blocks so that each
block's edge count per src-half (h row index split at SPLIT for the int16
gather index limit) fits a uniform per-block group budget.  Edge h[src] rows
are fetched with gpsimd.dma_gather; the scatter-add into the owning block is
a TensorEngine matmul against per-group one-hot matrices built on DVE with
an is_equal compare against an iota row.  Everything downstream runs in the
feature-on-partition ("transposed") orientation so no activation transposes
are needed except one PE transpose of the core's own h slab.
"""

import sys

for _p in ("/opt/trn_rl_repo",):
    if _p not in sys.path:
        sys.path.append(_p)

import numpy as np
import ml_dtypes

from concourse import bacc, bass, mybir, tile
from concourse import library_config
from concourse.masks import make_identity
from concourse.bass_utils import run_bass_kernel_spmd

P = 128
D = 256  # feature dim (fixed by problem)
NCORES = 8
BF16 = mybir.dt.bfloat16
F32 = mybir.dt.float32
I16 = mybir.dt.int16
np_bf16 = ml_dtypes.bfloat16


# ----------------------------------------------------------------------------
# Host-side sharding / packing (index work only -- no float math on h)
# ----------------------------------------------------------------------------

def _pack_nodes(deg_lo, deg_hi, n_nodes, ncores, nblk, tail_reserve):
    """Assign nodes to ncores*nblk bins of <=128 nodes, balancing per-bin
    (lo, hi) edge sums.  Zero-degree nodes are NOT passed here.
    tail_reserve[b] = slots reserved in bin b (for zero-deg + dummy tail).
    Returns bin_of_node-ish arrays: lists of node ids per bin."""
    nbins = ncores * nblk
    deg = deg_lo + deg_hi
    order = np.argsort(-deg, kind="stable")
    bins_lo = np.zeros(nbins, np.int64)
    bins_hi = np.zeros(nbins, np.int64)
    bins_cnt = np.zeros(nbins, np.int64)
    cap_cnt = np.full(nbins, P, np.int64) - tail_reserve
    members = [[] for _ in range(nbins)]
    dl = deg_lo[order]
    dh = deg_hi[order]
    for i, n in enumerate(order):
        lo = bins_lo + dl[i]
        hi = bins_hi + dh[i]
        score = np.maximum(lo, hi).astype(np.float64)
        score[bins_cnt >= cap_cnt] = np.inf
        b = int(np.argmin(score))
        members[b].append(int(n))
        bins_lo[b] += dl[i]
        bins_hi[b] += dh[i]
        bins_cnt[b] += 1
    return members, bins_lo, bins_hi


def host_prep(h, rel_emb, weight_neighbor, loop_weight, evolve_loop_weight,
              src, dst, etype, ncores=NCORES):
    """Build all per-core device input arrays + the structural metadata the
    graph builder needs.  Only integer/index work plus array slicing here."""
    n_nodes, d = h.shape
    assert d == D
    n_rel = rel_emb.shape[0]
    e = src.shape[0]
    src = np.asarray(src).astype(np.int64)
    dst = np.asarray(dst).astype(np.int64)
    etype = np.asarray(etype).astype(np.int64)

    split = min((n_nodes + 1) // 2, 32000)
    nblk = -(-n_nodes // (ncores * P))  # ceil
    # keep (nblk - ls) % 4 == 0 so full 512-wide output tiles align; ls >= 1
    ls = 1 + (nblk - 1) % 4
    slots = nblk * P

    e_half = (src >= split).astype(np.int64)
    deg_lo = np.bincount(dst[e_half == 0], minlength=n_nodes)
    deg_hi = np.bincount(dst[e_half == 1], minlength=n_nodes)
    deg = deg_lo + deg_hi

    zz = np.where(deg == 0)[0]
    nz = np.where(deg > 0)[0]
    n_dummy = ncores * slots - n_nodes
    # distribute zero-deg nodes round-robin over cores' tails
    z_per_core = [zz[c::ncores] for c in range(ncores)]
    tail_len = np.array([len(z) for z in z_per_core], np.int64)
    # dummies: n_dummy total, split as evenly as possible
    dummy_per_core = np.full(ncores, n_dummy // ncores, np.int64)
    dummy_per_core[: n_dummy % ncores] += 1
    tail_total = tail_len + dummy_per_core
    assert np.all(tail_total <= ls * P), (
        f"zero-degree tail {tail_total} exceeds select region {ls * P}"
    )

    nbins = ncores * nblk
    tail_reserve = np.zeros(nbins, np.int64)
    # reserve at the END of each core's bin range
    for c in range(ncores):
        rem = int(tail_total[c])
        b = nblk - 1
        while rem > 0:
            take = min(rem, P)
            tail_reserve[c * nblk + b] = take
            rem -= take
            b -= 1

    members, _, _ = _pack_nodes(
        deg_lo[nz], deg_hi[nz], len(nz), ncores, nblk, tail_reserve
    )
    members = [[nz[i] for i in m] for m in members]

    # append tails: zero-deg nodes then dummies (node id -1)
    for c in range(ncores):
        tail_nodes = list(z_per_core[c]) + [-1] * int(dummy_per_core[c])
        b = nblk - 1
        while tail_nodes:
            room = P - len(members[c * nblk + b])
            members[c * nblk + b].extend(tail_nodes[:room])
            tail_nodes = tail_nodes[room:]
            b -= 1

    node_of_slot = np.full((ncores, slots), -1, np.int64)
    for c in range(ncores):
        for b in range(nblk):
            m = members[c * nblk + b]
            assert len(m) <= P
            node_of_slot[c, b * P : b * P + len(m)] = m

    slot_of_node = np.full(n_nodes, -1, np.int64)
    flat = node_of_slot.reshape(-1)
    valid = flat >= 0
    slot_of_node[flat[valid]] = np.arange(ncores * slots)[valid]
    assert np.all(slot_of_node >= 0)

    gslot = slot_of_node[dst]           # global slot of each edge's dst
    e_core = gslot // slots
    e_local = gslot % slots
    e_blk = e_local // P
    e_lane = e_local % P

    # per-(core, blk, half) counts -> uniform group caps
    cnt = np.zeros((ncores, nblk, 2), np.int64)
    np.add.at(cnt, (e_core, e_blk, e_half), 1)
    cap_lo = max(1, int(-(-cnt[:, :, 0].max() // P)))
    cap_hi = max(1, int(-(-cnt[:, :, 1].max() // P)))

    arena_slots = (nblk * cap_lo * P, nblk * cap_hi * P)
    gidx = [np.zeros((ncores, arena_slots[a]), np.int64) for a in range(2)]
    dloc = [np.full((ncores, arena_slots[a]), 200, np.int64) for a in range(2)]

    order = np.lexsort((e_half, e_blk, e_core))
    sc, sb, sh = e_core[order], e_blk[order], e_half[order]
    slane = e_lane[order]
    ssrc = src[order]
    # segment starts over (core, blk, half)
    key = (sc * nblk + sb) * 2 + sh
    seg_starts = np.searchsorted(key, np.arange(ncores * nblk * 2))
    seg_ends = np.append(seg_starts[1:], e)
    cap_arr = (cap_lo * P, cap_hi * P)
    for c in range(ncores):
        for b in range(nblk):
            for a in range(2):
                k = (c * nblk + b) * 2 + a
                s0, s1 = seg_starts[k], seg_ends[k]
                cnt_e = s1 - s0
                assert cnt_e <= cap_arr[a]
                base = b * cap_arr[a]
                gidx[a][c, base : base + cnt_e] = ssrc[s0:s1] - (split if a else 0)
                dloc[a][c, base : base + cnt_e] = slane[s0:s1]

    # C^T: counts of (slot, etype) per core -> [n_rel, slots]
    ct = np.zeros((ncores, n_rel, slots), np.float32)
    np.add.at(ct, (e_core, etype, e_local), 1.0)

    # own h rows, pre-transposed [D, slots] (dummy slots -> zeros)
    h_ownT = np.zeros((ncores, D, slots), np.float32)
    for c in range(ncores):
        ns = node_of_slot[c]
        v = ns >= 0
        h_ownT[c][:, v] = h[ns[v]].T

    # select masks for the tail region (ls blocks): 1 -> Lw, 0 -> Ew
    z_needed = len(zz) > 0
    selm = np.ones((ncores, ls * P), np.float32)
    for c in range(ncores):
        ns = node_of_slot[c, (nblk - ls) * P :]
        selm[c] = ((ns >= 0) & (deg[np.maximum(ns, 0)] > 0)).astype(np.float32)

    # wrap gather indices per gather-call chunk of GPC groups (<=1024 idxs,
    # the HW per-dma_gather limit at 1KB rows) -> [16, n/16] tiled to 128 rows
    gpc = 8

    def wrap_idx(arr_a, cap):
        out = []
        ng = nblk * cap
        for c in range(ncores):
            rows = []
            for g0 in range(0, ng, gpc):
                gl = min(gpc, ng - g0)
                seg = arr_a[c, g0 * P : (g0 + gl) * P]
                w = seg.reshape(-1, 16).T  # [16, n/16]
                rows.append(np.tile(w, (8, 1)))
            out.append(np.concatenate(rows, axis=1))
        return np.stack(out).astype(np.int16)

    gidx_w = [wrap_idx(gidx[a], (cap_lo, cap_hi)[a]) for a in range(2)]
    # one-hot matrices, built host-side (pure index data):
    # oh[a][c][lane, group*P + d] = 1.0 iff edge at slot (group*P + lane) has
    # dst-lane d; pad slots (dloc=200) give all-zero rows.
    oh_w = []
    for a in range(2):
        per_core = []
        for c in range(ncores):
            dl = dloc[a][c].reshape(-1, P)            # [ngroups, lane] -> dstlane
            ng = dl.shape[0]
            ohm = np.zeros((ng, P, P), np.float32)    # [group, lane, d]
            gi, li = np.nonzero(dl < P)
            ohm[gi, li, dl[gi, li]] = 1.0
            # -> [lane(part), group*P + d]
            per_core.append(ohm.transpose(1, 0, 2).reshape(P, ng * P))
        oh_w.append(np.stack(per_core).astype(ml_dtypes.float8_e4m3))

    n8 = 0
    in_maps = []
    for c in range(ncores):
        in_maps.append({
            "h_lo": np.ascontiguousarray(h[:split]).astype(np.float32),
            "h_hi": np.ascontiguousarray(h[split:]).astype(np.float32),
            "h_ownt": h_ownT[c],
            "gidx_lo": gidx_w[0][c],
            "gidx_hi": gidx_w[1][c],
            "oh_lo": oh_w[0][c],
            "oh_hi": oh_w[1][c],
            "ct": ct[c].astype(np_bf16),
            "relt": np.ascontiguousarray(rel_emb.T).astype(np.float32),
            "w_n": weight_neighbor.astype(np.float32),
            "w_l": loop_weight.astype(np.float32),
            "w_e": evolve_loop_weight.astype(np.float32),
            "selm": np.tile(selm[c][None, :], (P, 1)).astype(np_bf16),
        })

    meta = dict(
        n_nodes=n_nodes, n_rel=n_rel, split=split, nblk=nblk, ls=ls,
        slots=slots, cap_lo=cap_lo, cap_hi=cap_hi, gpc=gpc, n8=n8,
        z_needed=bool(z_needed), node_of_slot=node_of_slot,
    )
    return in_maps, meta


# ----------------------------------------------------------------------------
# Device graph
# ----------------------------------------------------------------------------

def build_graph(meta, ncores=NCORES):
    FP8 = mybir.dt.float8e4
    n_nodes = meta["n_nodes"]
    n_rel = meta["n_rel"]
    split = meta["split"]
    nblk = meta["nblk"]
    ls = meta["ls"]
    slots = meta["slots"]
    cap_lo = meta["cap_lo"]
    cap_hi = meta["cap_hi"]
    gpc = meta["gpc"]
    n8 = meta["n8"]
    z_needed = meta["z_needed"]
    caps = (cap_lo, cap_hi)

    nc = bacc.Bacc("TRN2", target_bir_lowering=False, debug=False,
                   num_devices=ncores, num_swdge_queues=4)

    dt = nc.dram_tensor
    h_lo = dt("h_lo", [split, D], F32, kind="ExternalInput")
    h_hi = dt("h_hi", [n_nodes - split, D], F32, kind="ExternalInput")
    h_ownt = dt("h_ownt", [D, slots], F32, kind="ExternalInput")
    gidx = [dt("gidx_lo", [P, nblk * cap_lo * 8], I16, kind="ExternalInput"),
            dt("gidx_hi", [P, nblk * cap_hi * 8], I16, kind="ExternalInput")]
    FP8 = mybir.dt.float8e4
    ohd = [dt("oh_lo", [P, nblk * cap_lo * P], FP8, kind="ExternalInput"),
           dt("oh_hi", [P, nblk * cap_hi * P], FP8, kind="ExternalInput")]
    ct = dt("ct", [n_rel, slots], BF16, kind="ExternalInput")
    relt = dt("relt", [D, n_rel], F32, kind="ExternalInput")
    w_n = dt("w_n", [D, D], F32, kind="ExternalInput")
    w_l = dt("w_l", [D, D], F32, kind="ExternalInput")
    w_e = dt("w_e", [D, D], F32, kind="ExternalInput")
    selm = dt("selm", [P, ls * P], BF16, kind="ExternalInput")
    out = dt("out", [2, P, slots], BF16, kind="ExternalOutput")

    rel_k = [(k, min(P, n_rel - k)) for k in range(0, n_rel, P)]

    with tile.TileContext(nc) as tc:
        with (
            tc.tile_pool(name="dram", bufs=1, space="DRAM") as dp,
            tc.tile_pool(name="persist", bufs=1) as pp,
            tc.tile_pool(name="gather", bufs=3) as gp,
            tc.tile_pool(name="gbf", bufs=3) as gbp,
            tc.tile_pool(name="oh", bufs=3) as ohp,
            tc.tile_pool(name="wrk", bufs=2) as wp,
            tc.tile_pool(name="epi", bufs=3) as ep,
            tc.tile_pool(name="psum", bufs=8, space="PSUM") as psp,
        ):
            nc.gpsimd.load_library(library_config.mlp)
            h_src = [h_lo.ap(), h_hi.ap()]
            gidx_t = [pp.tile([P, nblk * caps[a] * 8], I16, tag=f"gidx{a}",
                              name=f"gidx_t{a}") for a in range(2)]
            for a in range(2):
                nc.sync.dma_start(gidx_t[a][:], gidx[a].ap())
            selm_t = pp.tile([P, ls * P], BF16, tag="selm")
            nc.sync.dma_start(selm_t[:], selm.ap())
            ct_t = [pp.tile([kn, slots], BF16, tag=f"ct{k}", name=f"ct_t{k}")
                    for k, kn in rel_k]
            for i, (k, kn) in enumerate(rel_k):
                nc.sync.dma_start(ct_t[i][:], ct.ap()[k:k + kn, :])

            # weights: load f32, cast to bf16 [K=D rows on partitions]
            def load_w_bf(w, tag):
                tiles = []
                for kc in range(D // P):
                    wf = wp.tile([P, D], F32, tag="wf32")
                    nc.sync.dma_start(wf[:], w.ap()[kc * P:(kc + 1) * P, :])
                    wb = pp.tile([P, D], BF16, tag=f"{tag}{kc}")
                    nc.vector.tensor_copy(wb[:], wf[:])
                    tiles.append(wb)
                return tiles

            w_n_t = load_w_bf(w_n, "wn")
            w_l_t = load_w_bf(w_l, "wl")
            w_e_t = load_w_bf(w_e, "we") if z_needed else None

            # rel_w = rel_emb @ Wn on-device from host-side rel^T layout
            relT = [pp.tile([P, n_rel], BF16, tag=f"relT{kc}",
                            name=f"relT{kc}") for kc in range(2)]
            for kc in range(2):
                rf = wp.tile([P, n_rel], F32, tag="relf32")
                nc.sync.dma_start(rf[:], relt.ap()[kc * P:(kc + 1) * P, :])
                nc.vector.tensor_copy(relT[kc][:], rf[:])
            relw_t = []
            for ki, (k, kn) in enumerate(rel_k):
                ps = psp.tile([P, D], F32, tag="ps")
                for kc in range(2):
                    nc.tensor.matmul(ps[:kn, :], relT[kc][:, k:k + kn],
                                     w_n_t[kc][:], start=(kc == 0),
                                     stop=(kc == 1))
                rw = pp.tile([kn, D], BF16, tag=f"relw{k}", name=f"relw{k}")
                nc.vector.tensor_copy(rw[:], ps[:kn, :])
                relw_t.append(rw)

            import os as _os
            _skip = set(_os.environ.get("KSKIP", "").split(","))

            # ---- h_ownT: host ships [D, slots] f32; load + cast to bf16 ------
            ht = pp.tile([P, 2, slots], BF16, tag="ht")
            if "ht" in _skip:
                nc.vector.memset(ht[:], 0.0)
            for fh in ([] if "ht" in _skip else range(2)):
                for mb in range(0, nblk, 8):
                    w_ = min(8 * P, slots - mb * P)
                    hf = wp.tile([P, 8 * P], F32, tag="hf32")
                    nc.sync.dma_start(
                        hf[:, :w_],
                        h_ownt.ap()[fh * P:(fh + 1) * P, mb * P:mb * P + w_])
                    nc.scalar.activation(ht[:, fh, mb * P:mb * P + w_],
                                          hf[:, :w_],
                                          mybir.ActivationFunctionType.Identity)

            # ---- S^T accumulation: gather + one-hot matmuls ------------------
            st = pp.tile([P, 2, slots], BF16, tag="st")  # [feat-half][f, slot]
            if "units" in _skip:
                nc.vector.memset(st[:], 0.0)

            # ---- epilogue (emitted interleaved: tile t as soon as its 4 ------
            # blocks of st have been flushed, so it overlaps the gathers)
            ntiles = [(t * 512, 512) for t in range((nblk - ls) * P // 512)]
            ntiles += [((nblk - ls) * P + t * P, P) for t in range(ls)]
            sel_from = (nblk - ls) * P

            def emit_tile(n0, nw):
                for fh in range(2):
                    in_sel = z_needed and n0 >= sel_from
                    agg = psp.tile([P, nw], F32, tag="ps", name=f"agg{n0}_{fh}")
                    nmm = 2 + len(rel_k)
                    i = 0
                    for kc in range(2):
                        nc.tensor.matmul(
                            agg[:], w_n_t[kc][:, fh * P:(fh + 1) * P],
                            st[:, kc, n0:n0 + nw],
                            start=(i == 0), stop=(i == nmm - 1))
                        i += 1
                    for ki, (k, kn) in enumerate(rel_k):
                        nc.tensor.matmul(
                            agg[:], relw_t[ki][:, fh * P:(fh + 1) * P],
                            ct_t[ki][:, n0:n0 + nw],
                            start=(i == 0), stop=(i == nmm - 1))
                        i += 1
                    ot = ep.tile([P, nw], BF16, tag="ot", name=f"ot{n0}_{fh}")
                    nc.scalar.activation(ot[:], agg[:],
                                         mybir.ActivationFunctionType.Relu)
                    # reuse the agg bank for the loop-message chain (start=True
                    # resets it; Tile orders it after the relu's read)
                    lp = agg
                    for kc in range(2):
                        nc.tensor.matmul(
                            lp[:], w_l_t[kc][:, fh * P:(fh + 1) * P],
                            ht[:, kc, n0:n0 + nw],
                            start=(kc == 0), stop=(kc == 1))
                    if not in_sel:
                        nc.vector.tensor_add(ot[:], ot[:], lp[:])
                    else:
                        ep_ps = psp.tile([P, nw], F32, tag="ps",
                                         name=f"eps{n0}_{fh}")
                        for kc in range(2):
                            nc.tensor.matmul(
                                ep_ps[:], w_e_t[kc][:, fh * P:(fh + 1) * P],
                                ht[:, kc, n0:n0 + nw],
                                start=(kc == 0), stop=(kc == 1))
                        lsb = ep.tile([P, nw], F32, tag="lsb",
                                      name=f"lsb{n0}_{fh}")
                        m0 = n0 - sel_from
                        nc.vector.tensor_tensor(
                            lsb[:], lp[:], selm_t[:, m0:m0 + nw],
                            op=mybir.AluOpType.mult)
                        nc.vector.tensor_add(ot[:], ot[:], lsb[:])
                        # (1 - m) * E = E - m*E
                        me = ep.tile([P, nw], F32, tag="me",
                                     name=f"me{n0}_{fh}")
                        nc.vector.tensor_tensor(
                            me[:], ep_ps[:], selm_t[:, m0:m0 + nw],
                            op=mybir.AluOpType.mult)
                        nc.vector.tensor_tensor(
                            me[:], ep_ps[:], me[:],
                            op=mybir.AluOpType.subtract)
                        nc.vector.tensor_add(ot[:], ot[:], me[:])
                    nc.sync.dma_start(out.ap()[fh, :, n0:n0 + nw], ot[:])
            pending_tiles = list(ntiles)

            # gather call k of arena a covers groups [k*gpc, k*gpc+gl);
            # rotate SWDGE queues: queue q runs on Q7 core pair {2q, 2q+1},
            # so consecutive calls on different queues overlap desc-gen.
            # SWDGE must carry ONLY these gathers so the round-robin DMASW
            # sem lanes each see a single queue.
            g_tiles = [{}, {}]
            qctr = [0]

            def ensure_call(a, call):
                if call in g_tiles[a]:
                    return
                cap = caps[a]
                ng = nblk * cap
                g0 = call * gpc
                gl = min(gpc, ng - g0)
                n_idx = gl * P
                gf = gp.tile([P, gpc, D], F32, tag=f"g{a}",
                             name=f"gf_{a}_{call}")
                nc.gpsimd.dma_gather(
                    gf[:, :gl, :], h_src[a],
                    gidx_t[a][:, g0 * 8:(g0 + gl) * 8],
                    n_idx, n_idx, D, queue_num=qctr[0] % 4,
                )
                qctr[0] += 1
                gb = gbp.tile([P, gpc, D], BF16, tag=f"gb{a}",
                              name=f"gb_{a}_{call}")
                nc.vector.tensor_copy(gb[:, :gl, :], gf[:, :gl, :])
                g_tiles[a][call] = gb

            for b in ([] if "units" in _skip else range(nblk)):
                for a in range(2):
                    for g in range(caps[a]):
                        ensure_call(a, (b * caps[a] + g) // gpc)
                ohs = []
                for a in range(2):
                    oh8 = ohp.tile([P, caps[a] * P], FP8, tag=f"oh8{a}",
                                   name=f"oh8_{a}_{b}")
                    nc.sync.dma_start(
                        oh8[:],
                        ohd[a].ap()[:, b * caps[a] * P:(b + 1) * caps[a] * P])
                    oh = ohp.tile([P, caps[a] * P], BF16, tag=f"oh{a}",
                                  name=f"oh_{a}_{b}")
                    nc.scalar.activation(oh[:], oh8[:],
                                         mybir.ActivationFunctionType.Identity)
                    ohs.append(oh)
                for fh in range(2):
                    ps = psp.tile([P, P], F32, tag="ps", name=f"ps_{b}_{fh}")
                    tot = caps[0] + caps[1]
                    i = 0
                    for a in range(2):
                        for g in range(caps[a]):
                            gg = b * caps[a] + g
                            gb = g_tiles[a][gg // gpc]
                            nc.tensor.matmul(
                                ps[:],
                                gb[:, gg % gpc, fh * P:(fh + 1) * P],
                                ohs[a][:, g * P:(g + 1) * P],
                                start=(i == 0), stop=(i == tot - 1),
                            )
                            i += 1
                    nc.scalar.activation(
                        st[:, fh, b * P:(b + 1) * P], ps[:],
                        mybir.ActivationFunctionType.Identity)
                while (pending_tiles and
                       pending_tiles[0][0] + pending_tiles[0][1]
                       <= (b + 1) * P):
                    n0_, nw_ = pending_tiles.pop(0)
                    emit_tile(n0_, nw_)

            for n0_, nw_ in pending_tiles:
                emit_tile(n0_, nw_)


    nc.compile()
    return nc


# ----------------------------------------------------------------------------
# Entry point
# ----------------------------------------------------------------------------

_CACHE = {}


def _run(inputs, ncores=NCORES, sim=False, trace=False):
    h = np.asarray(inputs["h"], np.float32)
    rel_emb = np.asarray(inputs["rel_emb"], np.float32)
    w_n = np.asarray(inputs["weight_neighbor"], np.float32)
    w_l = np.asarray(inputs["loop_weight"], np.float32)
    w_e = np.asarray(inputs["evolve_loop_weight"], np.float32)
    src = np.asarray(inputs["src"])
    dst = np.asarray(inputs["dst"])
    etype = np.asarray(inputs["etype"])

    in_maps, meta = host_prep(h, rel_emb, w_n, w_l, w_e, src, dst, etype,
                              ncores=ncores)

    key = (meta["n_nodes"], meta["n_rel"], meta["split"], meta["nblk"],
           meta["cap_lo"], meta["cap_hi"], meta["ls"], meta["z_needed"], ncores)
    if key not in _CACHE:
        _CACHE[key] = build_graph(meta, ncores=ncores)
    nc = _CACHE[key]

    res = run_bass_kernel_spmd(nc, in_maps, core_ids=list(range(ncores)),
                               trace=trace)

    n_nodes = meta["n_nodes"]
    slots = meta["slots"]
    node_of_slot = meta["node_of_slot"]
    out_full = np.zeros((n_nodes, D), np.float32)
    for c in range(ncores):
        oc = np.asarray(res.results[c]["out"])  # [2, P, slots]
        oc = oc.reshape(2 * P, slots)  # [D(feature), slots]
        ns = node_of_slot[c]
        v = ns >= 0
        out_full[ns[v]] = oc[:, v].T
    return out_full, res, meta


def kernel(**inputs) -> np.ndarray:
    out, _, _ = _run(inputs)
    return out.astype(np.float32)


if __name__ == "__main__":
    # tiny smoke test with a synthetic small graph
    rng = np.random.default_rng(0)
    N_, E_, R_ = 2048, 16384, 16
    inputs = dict(
        h=rng.standard_normal((N_, D), dtype=np.float32),
        rel_emb=(rng.standard_normal((R_, D)) * 0.1).astype(np.float32),
        weight_neighbor=rng.standard_normal((D, D), dtype=np.float32) * 0.05,
        a=np.zeros((3 * D, 1), np.float32),
        W3=np.zeros((3 * D, 3 * D), np.float32),
        loop_weight=rng.standard_normal((D, D), dtype=np.float32) * 0.05,
        evolve_loop_weight=rng.standard_normal((D, D), dtype=np.float32) * 0.05,
        src=rng.integers(0, N_, E_),
        dst=rng.integers(0, N_, E_),
        etype=rng.integers(0, R_, E_),
    )
    out, res, meta = _run({k: v for k, v in inputs.items()
                           if k not in ("a", "W3")})
    # numpy reference
    S = np.zeros((N_, D), np.float32)
    np.add.at(S, inputs["dst"], inputs["h"][inputs["src"]]
              + inputs["rel_emb"][inputs["etype"]])
    indeg = np.bincount(inputs["dst"], minlength=N_)
    loopm = np.where((indeg > 0)[:, None],
                     inputs["h"] @ inputs["loop_weight"],
                     inputs["h"] @ inputs["evolve_loop_weight"])
    ref = np.maximum(S @ inputs["weight_neighbor"], 0) + loopm
    err = np.abs(out - ref).max() / (np.abs(ref).max() + 1e-9)
    print("small-graph rel err:", err)


# revision 33
# speedup vs baseline: 2.0464x; 1.0201x over previous
"""AttentionRGCN layer on 8 Trainium2 NeuronCores (Bass/Tile).

Math notes (vs the jax reference):
  - alpha = exp(leaky_relu(score)) normalized over axis=1 of an [E,1] tensor
    is exactly 1.0 for every edge (x/x with finite positive x), so the whole
    attention branch (W3, a, score) drops out of the computation.
  - matmul commutes with segment_sum:
        segsum((h[src]+rel[etype]) @ Wn, dst) = segsum(h[src]+rel[etype], dst) @ Wn
    and segsum(rel[etype], dst) = C @ rel where C[n, r] counts (dst=n, etype=r)
    edges.  C is pure index data, so it is built host-side during sharding.
  So per node:  out = relu((S + C@rel) @ Wn) + (indeg>0 ? h@Lw : h@Ew)
  with S = segsum(h[src], dst).

Sharding: destination-node sharding.  Each core owns NBLK blocks of 128 node
slots; a host-side 2D bin-packing permutes nodes into blocks so that each
block's edge count per src-half (h row index split at SPLIT for the int16
gather index limit) fits a uniform per-block group budget.  Edge h[src] rows
are fetched with gpsimd.dma_gather; the scatter-add into the owning block is
a TensorEngine matmul against per-group one-hot matrices built on DVE with
an is_equal compare against an iota row.  Everything downstream runs in the
feature-on-partition ("transposed") orientation so no activation transposes
are needed except one PE transpose of the core's own h slab.
"""

import sys

for _p in ("/opt/trn_rl_repo",):
    if _p not in sys.path:
        sys.path.append(_p)

import numpy as np
import ml_dtypes

from concourse import bacc, bass, mybir, tile
from concourse import library_config
from concourse.masks import make_identity
from concourse.bass_utils import run_bass_kernel_spmd

P = 128
D = 256  # feature dim (fixed by problem)
NCORES = 8
BF16 = mybir.dt.bfloat16
F32 = mybir.dt.float32
I16 = mybir.dt.int16
np_bf16 = ml_dtypes.bfloat16


# ----------------------------------------------------------------------------
# Host-side sharding / packing (index work only -- no float math on h)
# ----------------------------------------------------------------------------

def _pack_nodes(deg_lo, deg_hi, n_nodes, ncores, nblk, tail_reserve):
    """Assign nodes to ncores*nblk bins of <=128 nodes, balancing per-bin
    (lo, hi) edge sums.  Zero-degree nodes are NOT passed here.
    tail_reserve[b] = slots reserved in bin b (for zero-deg + dummy tail).
    Returns bin_of_node-ish arrays: lists of node ids per bin."""
    nbins = ncores * nblk
    deg = deg_lo + deg_hi
    order = np.argsort(-deg, kind="stable")
    bins_lo = np.zeros(nbins, np.int64)
    bins_hi = np.zeros(nbins, np.int64)
    bins_cnt = np.zeros(nbins, np.int64)
    cap_cnt = np.full(nbins, P, np.int64) - tail_reserve
    members = [[] for _ in range(nbins)]
    dl = deg_lo[order]
    dh = deg_hi[order]
    for i, n in enumerate(order):
        lo = bins_lo + dl[i]
        hi = bins_hi + dh[i]
        score = np.maximum(lo, hi).astype(np.float64)
        score[bins_cnt >= cap_cnt] = np.inf
        b = int(np.argmin(score))
        members[b].append(int(n))
        bins_lo[b] += dl[i]
        bins_hi[b] += dh[i]
        bins_cnt[b] += 1
    return members, bins_lo, bins_hi


def host_prep(h, rel_emb, weight_neighbor, loop_weight, evolve_loop_weight,
              src, dst, etype, ncores=NCORES):
    """Build all per-core device input arrays + the structural metadata the
    graph builder needs.  Only integer/index work plus array slicing here."""
    n_nodes, d = h.shape
    assert d == D
    n_rel = rel_emb.shape[0]
    e = src.shape[0]
    src = np.asarray(src).astype(np.int64)
    dst = np.asarray(dst).astype(np.int64)
    etype = np.asarray(etype).astype(np.int64)

    split = min((n_nodes + 1) // 2, 32000)
    nblk = -(-n_nodes // (ncores * P))  # ceil
    # keep (nblk - ls) % 4 == 0 so full 512-wide output tiles align; ls >= 1
    ls = 1 + (nblk - 1) % 4
    slots = nblk * P

    e_half = (src >= split).astype(np.int64)
    deg_lo = np.bincount(dst[e_half == 0], minlength=n_nodes)
    deg_hi = np.bincount(dst[e_half == 1], minlength=n_nodes)
    deg = deg_lo + deg_hi

    zz = np.where(deg == 0)[0]
    nz = np.where(deg > 0)[0]
    n_dummy = ncores * slots - n_nodes
    # distribute zero-deg nodes round-robin over cores' tails
    z_per_core = [zz[c::ncores] for c in range(ncores)]
    tail_len = np.array([len(z) for z in z_per_core], np.int64)
    # dummies: n_dummy total, split as evenly as possible
    dummy_per_core = np.full(ncores, n_dummy // ncores, np.int64)
    dummy_per_core[: n_dummy % ncores] += 1
    tail_total = tail_len + dummy_per_core
    assert np.all(tail_total <= ls * P), (
        f"zero-degree tail {tail_total} exceeds select region {ls * P}"
    )

    nbins = ncores * nblk
    tail_reserve = np.zeros(nbins, np.int64)
    # reserve at the END of each core's bin range
    for c in range(ncores):
        rem = int(tail_total[c])
        b = nblk - 1
        while rem > 0:
            take = min(rem, P)
            tail_reserve[c * nblk + b] = take
            rem -= take
            b -= 1

    members, _, _ = _pack_nodes(
        deg_lo[nz], deg_hi[nz], len(nz), ncores, nblk, tail_reserve
    )
    members = [[nz[i] for i in m] for m in members]

    # append tails: zero-deg nodes then dummies (node id -1)
    for c in range(ncores):
        tail_nodes = list(z_per_core[c]) + [-1] * int(dummy_per_core[c])
        b = nblk - 1
        while tail_nodes:
            room = P - len(members[c * nblk + b])
            members[c * nblk + b].extend(tail_nodes[:room])
            tail_nodes = tail_nodes[room:]
            b -= 1

    node_of_slot = np.full((ncores, slots), -1, np.int64)
    for c in range(ncores):
        for b in range(nblk):
            m = members[c * nblk + b]
            assert len(m) <= P
            node_of_slot[c, b * P : b * P + len(m)] = m

    slot_of_node = np.full(n_nodes, -1, np.int64)
    flat = node_of_slot.reshape(-1)
    valid = flat >= 0
    slot_of_node[flat[valid]] = np.arange(ncores * slots)[valid]
    assert np.all(slot_of_node >= 0)

    gslot = slot_of_node[dst]           # global slot of each edge's dst
    e_core = gslot // slots
    e_local = gslot % slots
    e_blk = e_local // P
    e_lane = e_local % P

    # per-(core, blk, half) counts -> uniform group caps
    cnt = np.zeros((ncores, nblk, 2), np.int64)
    np.add.at(cnt, (e_core, e_blk, e_half), 1)
    cap_lo = max(1, int(-(-cnt[:, :, 0].max() // P)))
    cap_hi = max(1, int(-(-cnt[:, :, 1].max() // P)))

    arena_slots = (nblk * cap_lo * P, nblk * cap_hi * P)
    gidx = [np.zeros((ncores, arena_slots[a]), np.int64) for a in range(2)]
    dloc = [np.full((ncores, arena_slots[a]), 200, np.int64) for a in range(2)]

    order = np.lexsort((e_half, e_blk, e_core))
    sc, sb, sh = e_core[order], e_blk[order], e_half[order]
    slane = e_lane[order]
    ssrc = src[order]
    # segment starts over (core, blk, half)
    key = (sc * nblk + sb) * 2 + sh
    seg_starts = np.searchsorted(key, np.arange(ncores * nblk * 2))
    seg_ends = np.append(seg_starts[1:], e)
    cap_arr = (cap_lo * P, cap_hi * P)
    for c in range(ncores):
        for b in range(nblk):
            for a in range(2):
                k = (c * nblk + b) * 2 + a
                s0, s1 = seg_starts[k], seg_ends[k]
                cnt_e = s1 - s0
                assert cnt_e <= cap_arr[a]
                base = b * cap_arr[a]
                gidx[a][c, base : base + cnt_e] = ssrc[s0:s1] - (split if a else 0)
                dloc[a][c, base : base + cnt_e] = slane[s0:s1]

    # C^T: counts of (slot, etype) per core -> [n_rel, slots]
    ct = np.zeros((ncores, n_rel, slots), np.float32)
    np.add.at(ct, (e_core, etype, e_local), 1.0)

    # own h rows, pre-transposed [D, slots] (dummy slots -> zeros)
    h_ownT = np.zeros((ncores, D, slots), np.float32)
    for c in range(ncores):
        ns = node_of_slot[c]
        v = ns >= 0
        h_ownT[c][:, v] = h[ns[v]].T

    # select masks for the tail region (ls blocks): 1 -> Lw, 0 -> Ew
    z_needed = len(zz) > 0
    selm = np.ones((ncores, ls * P), np.float32)
    for c in range(ncores):
        ns = node_of_slot[c, (nblk - ls) * P :]
        selm[c] = ((ns >= 0) & (deg[np.maximum(ns, 0)] > 0)).astype(np.float32)

    # wrap gather indices per gather-call chunk of GPC groups (<=1024 idxs,
    # the HW per-dma_gather limit at 1KB rows) -> [16, n/16] tiled to 128 rows
    gpc = 8

    def wrap_idx(arr_a, cap):
        out = []
        ng = nblk * cap
        for c in range(ncores):
            rows = []
            for g0 in range(0, ng, gpc):
                gl = min(gpc, ng - g0)
                seg = arr_a[c, g0 * P : (g0 + gl) * P]
                w = seg.reshape(-1, 16).T  # [16, n/16]
                rows.append(np.tile(w, (8, 1)))
            out.append(np.concatenate(rows, axis=1))
        return np.stack(out).astype(np.int16)

    gidx_w = [wrap_idx(gidx[a], (cap_lo, cap_hi)[a]) for a in range(2)]
    # one-hot matrices, built host-side (pure index data):
    # oh[a][c][lane, group*P + d] = 1.0 iff edge at slot (group*P + lane) has
    # dst-lane d; pad slots (dloc=200) give all-zero rows.
    oh_w = []
    for a in range(2):
        per_core = []
        for c in range(ncores):
            dl = dloc[a][c].reshape(-1, P)            # [ngroups, lane] -> dstlane
            ng = dl.shape[0]
            ohm = np.zeros((ng, P, P), np.float32)    # [group, lane, d]
            gi, li = np.nonzero(dl < P)
            ohm[gi, li, dl[gi, li]] = 1.0
            # -> [lane(part), group*P + d]
            per_core.append(ohm.transpose(1, 0, 2).reshape(P, ng * P))
        oh_w.append(np.stack(per_core).astype(ml_dtypes.float8_e4m3))

    n8 = 0
    in_maps = []
    for c in range(ncores):
        in_maps.append({
            "h_lo": np.ascontiguousarray(h[:split]).astype(np.float32),
            "h_hi": np.ascontiguousarray(h[split:]).astype(np.float32),
            "h_ownt": h_ownT[c],
            "gidx_lo": gidx_w[0][c],
            "gidx_hi": gidx_w[1][c],
            "oh_lo": oh_w[0][c],
            "oh_hi": oh_w[1][c],
            "ct": ct[c].astype(np_bf16),
            "relt": np.ascontiguousarray(rel_emb.T).astype(np.float32),
            "w_n": weight_neighbor.astype(np.float32),
            "w_l": loop_weight.astype(np.float32),
            "w_e": evolve_loop_weight.astype(np.float32),
            "selm": np.tile(selm[c][None, :], (P, 1)).astype(np_bf16),
        })

    meta = dict(
        n_nodes=n_nodes, n_rel=n_rel, split=split, nblk=nblk, ls=ls,
        slots=slots, cap_lo=cap_lo, cap_hi=cap_hi, gpc=gpc, n8=n8,
        z_needed=bool(z_needed), node_of_slot=node_of_slot,
    )
    return in_maps, meta


# ----------------------------------------------------------------------------
# Device graph
# ----------------------------------------------------------------------------

def build_graph(meta, ncores=NCORES):
    FP8 = mybir.dt.float8e4
    n_nodes = meta["n_nodes"]
    n_rel = meta["n_rel"]
    split = meta["split"]
    nblk = meta["nblk"]
    ls = meta["ls"]
    slots = meta["slots"]
    cap_lo = meta["cap_lo"]
    cap_hi = meta["cap_hi"]
    gpc = meta["gpc"]
    n8 = meta["n8"]
    z_needed = meta["z_needed"]
    caps = (cap_lo, cap_hi)

    nc = bacc.Bacc("TRN2", target_bir_lowering=False, debug=False,
                   num_devices=ncores, num_swdge_queues=4)

    dt = nc.dram_tensor
    h_lo = dt("h_lo", [split, D], F32, kind="ExternalInput")
    h_hi = dt("h_hi", [n_nodes - split, D], F32, kind="ExternalInput")
    h_ownt = dt("h_ownt", [D, slots], F32, kind="ExternalInput")
    gidx = [dt("gidx_lo", [P, nblk * cap_lo * 8], I16, kind="ExternalInput"),
            dt("gidx_hi", [P, nblk * cap_hi * 8], I16, kind="ExternalInput")]
    FP8 = mybir.dt.float8e4
    ohd = [dt("oh_lo", [P, nblk * cap_lo * P], FP8, kind="ExternalInput"),
           dt("oh_hi", [P, nblk * cap_hi * P], FP8, kind="ExternalInput")]
    ct = dt("ct", [n_rel, slots], BF16, kind="ExternalInput")
    relt = dt("relt", [D, n_rel], F32, kind="ExternalInput")
    w_n = dt("w_n", [D, D], F32, kind="ExternalInput")
    w_l = dt("w_l", [D, D], F32, kind="ExternalInput")
    w_e = dt("w_e", [D, D], F32, kind="ExternalInput")
    selm = dt("selm", [P, ls * P], BF16, kind="ExternalInput")
    out = dt("out", [2, P, slots], BF16, kind="ExternalOutput")

    rel_k = [(k, min(P, n_rel - k)) for k in range(0, n_rel, P)]

    with tile.TileContext(nc) as tc:
        with (
            tc.tile_pool(name="dram", bufs=1, space="DRAM") as dp,
            tc.tile_pool(name="persist", bufs=1) as pp,
            tc.tile_pool(name="gather", bufs=3) as gp,
            tc.tile_pool(name="gbf", bufs=3) as gbp,
            tc.tile_pool(name="oh", bufs=3) as ohp,
            tc.tile_pool(name="wrk", bufs=2) as wp,
            tc.tile_pool(name="epi", bufs=3) as ep,
            tc.tile_pool(name="psum", bufs=8, space="PSUM") as psp,
        ):
            nc.gpsimd.load_library(library_config.mlp)
            h_src = [h_lo.ap(), h_hi.ap()]
            gidx_t = [pp.tile([P, nblk * caps[a] * 8], I16, tag=f"gidx{a}",
                              name=f"gidx_t{a}") for a in range(2)]
            for a in range(2):
                nc.sync.dma_start(gidx_t[a][:], gidx[a].ap())

            import os as _os
            _skip = set(_os.environ.get("KSKIP", "").split(","))

            # gather call k of arena a covers groups [k*gpc, k*gpc+gl);
            # rotate SWDGE queues: queue q runs on Q7 core pair {2q, 2q+1},
            # so consecutive calls on different queues overlap desc-gen.
            # SWDGE must carry ONLY these gathers so the round-robin DMASW
            # sem lanes each see a single queue.
            g_tiles = [{}, {}]
            qctr = [0]

            def ensure_call(a, call):
                if call in g_tiles[a]:
                    return
                cap = caps[a]
                ng = nblk * cap
                g0 = call * gpc
                gl = min(gpc, ng - g0)
                n_idx = gl * P
                gf = gp.tile([P, gpc, D], F32, tag=f"g{a}",
                             name=f"gf_{a}_{call}")
                nc.gpsimd.dma_gather(
                    gf[:, :gl, :], h_src[a],
                    gidx_t[a][:, g0 * 8:(g0 + gl) * 8],
                    n_idx, n_idx, D, queue_num=qctr[0] % 4,
                )
                qctr[0] += 1
                gb = gbp.tile([P, gpc, D], BF16, tag=f"gb{a}",
                              name=f"gb_{a}_{call}")
                nc.vector.tensor_copy(gb[:, :gl, :], gf[:, :gl, :])
                g_tiles[a][call] = gb

            # start the SWDGE/Q7 gather stream immediately; every other
            # load below overlaps behind it
            if "units" not in _skip:
                for _c in range(2):
                    ensure_call(0, _c)
                    ensure_call(1, _c)

            selm_t = pp.tile([P, ls * P], BF16, tag="selm")
            nc.sync.dma_start(selm_t[:], selm.ap())
            ct_t = [pp.tile([kn, slots], BF16, tag=f"ct{k}", name=f"ct_t{k}")
                    for k, kn in rel_k]
            for i, (k, kn) in enumerate(rel_k):
                nc.sync.dma_start(ct_t[i][:], ct.ap()[k:k + kn, :])

            # weights: load f32, cast to bf16 [K=D rows on partitions]
            def load_w_bf(w, tag):
                tiles = []
                for kc in range(D // P):
                    wf = wp.tile([P, D], F32, tag="wf32")
                    nc.sync.dma_start(wf[:], w.ap()[kc * P:(kc + 1) * P, :])
                    wb = pp.tile([P, D], BF16, tag=f"{tag}{kc}")
                    nc.vector.tensor_copy(wb[:], wf[:])
                    tiles.append(wb)
                return tiles

            w_n_t = load_w_bf(w_n, "wn")
            w_l_t = load_w_bf(w_l, "wl")
            w_e_t = load_w_bf(w_e, "we") if z_needed else None

            # rel_w = rel_emb @ Wn on-device from host-side rel^T layout
            relT = [pp.tile([P, n_rel], BF16, tag=f"relT{kc}",
                            name=f"relT{kc}") for kc in range(2)]
            for kc in range(2):
                rf = wp.tile([P, n_rel], F32, tag="relf32")
                nc.sync.dma_start(rf[:], relt.ap()[kc * P:(kc + 1) * P, :])
                nc.vector.tensor_copy(relT[kc][:], rf[:])
            relw_t = []
            for ki, (k, kn) in enumerate(rel_k):
                ps = psp.tile([P, D], F32, tag="ps")
                for kc in range(2):
                    nc.tensor.matmul(ps[:kn, :], relT[kc][:, k:k + kn],
                                     w_n_t[kc][:], start=(kc == 0),
                                     stop=(kc == 1))
                rw = pp.tile([kn, D], BF16, tag=f"relw{k}", name=f"relw{k}")
                nc.vector.tensor_copy(rw[:], ps[:kn, :])
                relw_t.append(rw)

            # ---- h_ownT: host ships [D, slots] f32; load + cast to bf16 ------
            ht = pp.tile([P, 2, slots], BF16, tag="ht")
            if "ht" in _skip:
                nc.vector.memset(ht[:], 0.0)
            for fh in ([] if "ht" in _skip else range(2)):
                for mb in range(0, nblk, 8):
                    w_ = min(8 * P, slots - mb * P)
                    hf = wp.tile([P, 8 * P], F32, tag="hf32")
                    nc.sync.dma_start(
                        hf[:, :w_],
                        h_ownt.ap()[fh * P:(fh + 1) * P, mb * P:mb * P + w_])
                    nc.scalar.activation(ht[:, fh, mb * P:mb * P + w_],
                                          hf[:, :w_],
                                          mybir.ActivationFunctionType.Identity)

            # ---- S^T accumulation: gather + one-hot matmuls ------------------
            st = pp.tile([P, 2, slots], BF16, tag="st")  # [feat-half][f, slot]
            if "units" in _skip:
                nc.vector.memset(st[:], 0.0)

            # ---- epilogue (emitted interleaved: tile t as soon as its 4 ------
            # blocks of st have been flushed, so it overlaps the gathers)
            ntiles = [(t * 512, 512) for t in range((nblk - ls) * P // 512)]
            ntiles += [((nblk - ls) * P + t * P, P) for t in range(ls)]
            sel_from = (nblk - ls) * P

            def emit_tile(n0, nw):
                for fh in range(2):
                    in_sel = z_needed and n0 >= sel_from
                    agg = psp.tile([P, nw], F32, tag="ps", name=f"agg{n0}_{fh}")
                    nmm = 2 + len(rel_k)
                    i = 0
                    for kc in range(2):
                        nc.tensor.matmul(
                            agg[:], w_n_t[kc][:, fh * P:(fh + 1) * P],
                            st[:, kc, n0:n0 + nw],
                            start=(i == 0), stop=(i == nmm - 1))
                        i += 1
                    for ki, (k, kn) in enumerate(rel_k):
                        nc.tensor.matmul(
                            agg[:], relw_t[ki][:, fh * P:(fh + 1) * P],
                            ct_t[ki][:, n0:n0 + nw],
                            start=(i == 0), stop=(i == nmm - 1))
                        i += 1
                    ot = ep.tile([P, nw], BF16, tag="ot", name=f"ot{n0}_{fh}")
                    nc.scalar.activation(ot[:], agg[:],
                                         mybir.ActivationFunctionType.Relu)
                    # reuse the agg bank for the loop-message chain (start=True
                    # resets it; Tile orders it after the relu's read)
                    lp = agg
                    for kc in range(2):
                        nc.tensor.matmul(
                            lp[:], w_l_t[kc][:, fh * P:(fh + 1) * P],
                            ht[:, kc, n0:n0 + nw],
                            start=(kc == 0), stop=(kc == 1))
                    if not in_sel:
                        nc.vector.tensor_add(ot[:], ot[:], lp[:])
                    else:
                        ep_ps = psp.tile([P, nw], F32, tag="ps",
                                         name=f"eps{n0}_{fh}")
                        for kc in range(2):
                            nc.tensor.matmul(
                                ep_ps[:], w_e_t[kc][:, fh * P:(fh + 1) * P],
                                ht[:, kc, n0:n0 + nw],
                                start=(kc == 0), stop=(kc == 1))
                        lsb = ep.tile([P, nw], F32, tag="lsb",
                                      name=f"lsb{n0}_{fh}")
                        m0 = n0 - sel_from
                        nc.vector.tensor_tensor(
                            lsb[:], lp[:], selm_t[:, m0:m0 + nw],
                            op=mybir.AluOpType.mult)
                        nc.vector.tensor_add(ot[:], ot[:], lsb[:])
                        # (1 - m) * E = E - m*E
                        me = ep.tile([P, nw], F32, tag="me",
                                     name=f"me{n0}_{fh}")
                        nc.vector.tensor_tensor(
                            me[:], ep_ps[:], selm_t[:, m0:m0 + nw],
                            op=mybir.AluOpType.mult)
                        nc.vector.tensor_tensor(
                            me[:], ep_ps[:], me[:],
                            op=mybir.AluOpType.subtract)
                        nc.vector.tensor_add(ot[:], ot[:], me[:])
                    nc.sync.dma_start(out.ap()[fh, :, n0:n0 + nw], ot[:])
            pending_tiles = list(ntiles)

            for b in ([] if "units" in _skip else range(nblk)):
                for a in range(2):
                    for g in range(caps[a]):
                        ensure_call(a, (b * caps[a] + g) // gpc)
                ohs = []
                for a in range(2):
                    oh8 = ohp.tile([P, caps[a] * P], FP8, tag=f"oh8{a}",
                                   name=f"oh8_{a}_{b}")
                    nc.sync.dma_start(
                        oh8[:],
                        ohd[a].ap()[:, b * caps[a] * P:(b + 1) * caps[a] * P])
                    oh = ohp.tile([P, caps[a] * P], BF16, tag=f"oh{a}",
                                  name=f"oh_{a}_{b}")
                    nc.scalar.activation(oh[:], oh8[:],
                                         mybir.ActivationFunctionType.Identity)
                    ohs.append(oh)
                for fh in range(2):
                    ps = psp.tile([P, P], F32, tag="ps", name=f"ps_{b}_{fh}")
                    tot = caps[0] + caps[1]
                    i = 0
                    for a in range(2):
                        for g in range(caps[a]):
                            gg = b * caps[a] + g
                            gb = g_tiles[a][gg // gpc]
                            nc.tensor.matmul(
                                ps[:],
                                gb[:, gg % gpc, fh * P:(fh + 1) * P],
                                ohs[a][:, g * P:(g + 1) * P],
                                start=(i == 0), stop=(i == tot - 1),
                            )
                            i += 1
                    nc.scalar.activation(
                        st[:, fh, b * P:(b + 1) * P], ps[:],
                        mybir.ActivationFunctionType.Identity)
                while (pending_tiles and
                       pending_tiles[0][0] + pending_tiles[0][1]
                       <= (b + 1) * P):
                    n0_, nw_ = pending_tiles.pop(0)
                    emit_tile(n0_, nw_)

            for n0_, nw_ in pending_tiles:
                emit_tile(n0_, nw_)


    nc.compile()
    return nc


# ----------------------------------------------------------------------------
# Entry point
# ----------------------------------------------------------------------------

_CACHE = {}


def _run(inputs, ncores=NCORES, sim=False, trace=False):
    h = np.asarray(inputs["h"], np.float32)
    rel_emb = np.asarray(inputs["rel_emb"], np.float32)
    w_n = np.asarray(inputs["weight_neighbor"], np.float32)
    w_l = np.asarray(inputs["loop_weight"], np.float32)
    w_e = np.asarray(inputs["evolve_loop_weight"], np.float32)
    src = np.asarray(inputs["src"])
    dst = np.asarray(inputs["dst"])
    etype = np.asarray(inputs["etype"])

    in_maps, meta = host_prep(h, rel_emb, w_n, w_l, w_e, src, dst, etype,
                              ncores=ncores)

    key = (meta["n_nodes"], meta["n_rel"], meta["split"], meta["nblk"],
           meta["cap_lo"], meta["cap_hi"], meta["ls"], meta["z_needed"], ncores)
    if key not in _CACHE:
        _CACHE[key] = build_graph(meta, ncores=ncores)
    nc = _CACHE[key]

    res = run_bass_kernel_spmd(nc, in_maps, core_ids=list(range(ncores)),
                               trace=trace)

    n_nodes = meta["n_nodes"]
    slots = meta["slots"]
    node_of_slot = meta["node_of_slot"]
    out_full = np.zeros((n_nodes, D), np.float32)
    for c in range(ncores):
        oc = np.asarray(res.results[c]["out"])  # [2, P, slots]
        oc = oc.reshape(2 * P, slots)  # [D(feature), slots]
        ns = node_of_slot[c]
        v = ns >= 0
        out_full[ns[v]] = oc[:, v].T
    return out_full, res, meta


def kernel(**inputs) -> np.ndarray:
    out, _, _ = _run(inputs)
    return out.astype(np.float32)


if __name__ == "__main__":
    # tiny smoke test with a synthetic small graph
    rng = np.random.default_rng(0)
    N_, E_, R_ = 2048, 16384, 16
    inputs = dict(
        h=rng.standard_normal((N_, D), dtype=np.float32),
        rel_emb=(rng.standard_normal((R_, D)) * 0.1).astype(np.float32),
        weight_neighbor=rng.standard_normal((D, D), dtype=np.float32) * 0.05,
        a=np.zeros((3 * D, 1), np.float32),
        W3=np.zeros((3 * D, 3 * D), np.float32),
        loop_weight=rng.standard_normal((D, D), dtype=np.float32) * 0.05,
        evolve_loop_weight=rng.standard_normal((D, D), dtype=np.float32) * 0.05,
        src=rng.integers(0, N_, E_),
        dst=rng.integers(0, N_, E_),
        etype=rng.integers(0, R_, E_),
    )
    out, res, meta = _run({k: v for k, v in inputs.items()
                           if k not in ("a", "W3")})
    # numpy reference
    S = np.zeros((N_, D), np.float32)
    np.add.at(S, inputs["dst"], inputs["h"][inputs["src"]]
              + inputs["rel_emb"][inputs["etype"]])
    indeg = np.bincount(inputs["dst"], minlength=N_)
    loopm = np.where((indeg > 0)[:, None],
                     inputs["h"] @ inputs["loop_weight"],
                     inputs["h"] @ inputs["evolve_loop_weight"])
    ref = np.maximum(S @ inputs["weight_neighbor"], 0) + loopm
    err = np.abs(out - ref).max() / (np.abs(ref).max() + 1e-9)
    print("small-graph rel err:", err)


# revision 34
# speedup vs baseline: 2.0721x; 1.0125x over previous
"""AttentionRGCN layer on 8 Trainium2 NeuronCores (Bass/Tile).

Math notes (vs the jax reference):
  - alpha = exp(leaky_relu(score)) normalized over axis=1 of an [E,1] tensor
    is exactly 1.0 for every edge (x/x with finite positive x), so the whole
    attention branch (W3, a, score) drops out of the computation.
  - matmul commutes with segment_sum:
        segsum((h[src]+rel[etype]) @ Wn, dst) = segsum(h[src]+rel[etype], dst) @ Wn
    and segsum(rel[etype], dst) = C @ rel where C[n, r] counts (dst=n, etype=r)
    edges.  C is pure index data, so it is built host-side during sharding.
  So per node:  out = relu((S + C@rel) @ Wn) + (indeg>0 ? h@Lw : h@Ew)
  with S = segsum(h[src], dst).

Sharding: destination-node sharding.  Each core owns NBLK blocks of 128 node
slots; a host-side 2D bin-packing permutes nodes into blocks so that each
block's edge count per src-half (h row index split at SPLIT for the int16
gather index limit) fits a uniform per-block group budget.  Edge h[src] rows
are fetched with gpsimd.dma_gather; the scatter-add into the owning block is
a TensorEngine matmul against per-group one-hot matrices built on DVE with
an is_equal compare against an iota row.  Everything downstream runs in the
feature-on-partition ("transposed") orientation so no activation transposes
are needed except one PE transpose of the core's own h slab.
"""

import sys

for _p in ("/opt/trn_rl_repo",):
    if _p not in sys.path:
        sys.path.append(_p)

import numpy as np
import ml_dtypes

from concourse import bacc, bass, mybir, tile
from concourse import library_config
from concourse.masks import make_identity
from concourse.bass_utils import run_bass_kernel_spmd

P = 128
D = 256  # feature dim (fixed by problem)
NCORES = 8
BF16 = mybir.dt.bfloat16
F32 = mybir.dt.float32
I16 = mybir.dt.int16
np_bf16 = ml_dtypes.bfloat16


# ----------------------------------------------------------------------------
# Host-side sharding / packing (index work only -- no float math on h)
# ----------------------------------------------------------------------------

def _pack_nodes(deg_lo, deg_hi, n_nodes, ncores, nblk, tail_reserve):
    """Assign nodes to ncores*nblk bins of <=128 nodes, balancing per-bin
    (lo, hi) edge sums.  Zero-degree nodes are NOT passed here.
    tail_reserve[b] = slots reserved in bin b (for zero-deg + dummy tail).
    Returns bin_of_node-ish arrays: lists of node ids per bin."""
    nbins = ncores * nblk
    deg = deg_lo + deg_hi
    order = np.argsort(-deg, kind="stable")
    bins_lo = np.zeros(nbins, np.int64)
    bins_hi = np.zeros(nbins, np.int64)
    bins_cnt = np.zeros(nbins, np.int64)
    cap_cnt = np.full(nbins, P, np.int64) - tail_reserve
    members = [[] for _ in range(nbins)]
    dl = deg_lo[order]
    dh = deg_hi[order]
    for i, n in enumerate(order):
        lo = bins_lo + dl[i]
        hi = bins_hi + dh[i]
        score = np.maximum(lo, hi).astype(np.float64)
        score[bins_cnt >= cap_cnt] = np.inf
        b = int(np.argmin(score))
        members[b].append(int(n))
        bins_lo[b] += dl[i]
        bins_hi[b] += dh[i]
        bins_cnt[b] += 1
    return members, bins_lo, bins_hi


def host_prep(h, rel_emb, weight_neighbor, loop_weight, evolve_loop_weight,
              src, dst, etype, ncores=NCORES):
    """Build all per-core device input arrays + the structural metadata the
    graph builder needs.  Only integer/index work plus array slicing here."""
    n_nodes, d = h.shape
    assert d == D
    n_rel = rel_emb.shape[0]
    e = src.shape[0]
    src = np.asarray(src).astype(np.int64)
    dst = np.asarray(dst).astype(np.int64)
    etype = np.asarray(etype).astype(np.int64)

    split = min((n_nodes + 1) // 2, 32000)
    nblk = -(-n_nodes // (ncores * P))  # ceil
    # keep (nblk - ls) % 4 == 0 so full 512-wide output tiles align; ls >= 1
    ls = 1 + (nblk - 1) % 4
    slots = nblk * P

    e_half = (src >= split).astype(np.int64)
    deg_lo = np.bincount(dst[e_half == 0], minlength=n_nodes)
    deg_hi = np.bincount(dst[e_half == 1], minlength=n_nodes)
    deg = deg_lo + deg_hi

    zz = np.where(deg == 0)[0]
    nz = np.where(deg > 0)[0]
    n_dummy = ncores * slots - n_nodes
    # distribute zero-deg nodes round-robin over cores' tails
    z_per_core = [zz[c::ncores] for c in range(ncores)]
    tail_len = np.array([len(z) for z in z_per_core], np.int64)
    # dummies: n_dummy total, split as evenly as possible
    dummy_per_core = np.full(ncores, n_dummy // ncores, np.int64)
    dummy_per_core[: n_dummy % ncores] += 1
    tail_total = tail_len + dummy_per_core
    assert np.all(tail_total <= ls * P), (
        f"zero-degree tail {tail_total} exceeds select region {ls * P}"
    )

    nbins = ncores * nblk
    tail_reserve = np.zeros(nbins, np.int64)
    # reserve at the END of each core's bin range
    for c in range(ncores):
        rem = int(tail_total[c])
        b = nblk - 1
        while rem > 0:
            take = min(rem, P)
            tail_reserve[c * nblk + b] = take
            rem -= take
            b -= 1

    members, _, _ = _pack_nodes(
        deg_lo[nz], deg_hi[nz], len(nz), ncores, nblk, tail_reserve
    )
    members = [[nz[i] for i in m] for m in members]

    # append tails: zero-deg nodes then dummies (node id -1)
    for c in range(ncores):
        tail_nodes = list(z_per_core[c]) + [-1] * int(dummy_per_core[c])
        b = nblk - 1
        while tail_nodes:
            room = P - len(members[c * nblk + b])
            members[c * nblk + b].extend(tail_nodes[:room])
            tail_nodes = tail_nodes[room:]
            b -= 1

    node_of_slot = np.full((ncores, slots), -1, np.int64)
    for c in range(ncores):
        for b in range(nblk):
            m = members[c * nblk + b]
            assert len(m) <= P
            node_of_slot[c, b * P : b * P + len(m)] = m

    slot_of_node = np.full(n_nodes, -1, np.int64)
    flat = node_of_slot.reshape(-1)
    valid = flat >= 0
    slot_of_node[flat[valid]] = np.arange(ncores * slots)[valid]
    assert np.all(slot_of_node >= 0)

    gslot = slot_of_node[dst]           # global slot of each edge's dst
    e_core = gslot // slots
    e_local = gslot % slots
    e_blk = e_local // P
    e_lane = e_local % P

    # per-(core, blk, half) counts -> uniform group caps
    cnt = np.zeros((ncores, nblk, 2), np.int64)
    np.add.at(cnt, (e_core, e_blk, e_half), 1)
    cap_lo = max(1, int(-(-cnt[:, :, 0].max() // P)))
    cap_hi = max(1, int(-(-cnt[:, :, 1].max() // P)))

    arena_slots = (nblk * cap_lo * P, nblk * cap_hi * P)
    gidx = [np.zeros((ncores, arena_slots[a]), np.int64) for a in range(2)]
    dloc = [np.full((ncores, arena_slots[a]), 200, np.int64) for a in range(2)]

    order = np.lexsort((e_half, e_blk, e_core))
    sc, sb, sh = e_core[order], e_blk[order], e_half[order]
    slane = e_lane[order]
    ssrc = src[order]
    # segment starts over (core, blk, half)
    key = (sc * nblk + sb) * 2 + sh
    seg_starts = np.searchsorted(key, np.arange(ncores * nblk * 2))
    seg_ends = np.append(seg_starts[1:], e)
    cap_arr = (cap_lo * P, cap_hi * P)
    for c in range(ncores):
        for b in range(nblk):
            for a in range(2):
                k = (c * nblk + b) * 2 + a
                s0, s1 = seg_starts[k], seg_ends[k]
                cnt_e = s1 - s0
                assert cnt_e <= cap_arr[a]
                base = b * cap_arr[a]
                gidx[a][c, base : base + cnt_e] = ssrc[s0:s1] - (split if a else 0)
                dloc[a][c, base : base + cnt_e] = slane[s0:s1]

    # C^T: counts of (slot, etype) per core -> [n_rel, slots]
    ct = np.zeros((ncores, n_rel, slots), np.float32)
    np.add.at(ct, (e_core, etype, e_local), 1.0)

    # own h rows, pre-transposed [D, slots] (dummy slots -> zeros)
    h_ownT = np.zeros((ncores, D, slots), np.float32)
    for c in range(ncores):
        ns = node_of_slot[c]
        v = ns >= 0
        h_ownT[c][:, v] = h[ns[v]].T

    # select masks for the tail region (ls blocks): 1 -> Lw, 0 -> Ew
    z_needed = len(zz) > 0
    selm = np.ones((ncores, ls * P), np.float32)
    for c in range(ncores):
        ns = node_of_slot[c, (nblk - ls) * P :]
        selm[c] = ((ns >= 0) & (deg[np.maximum(ns, 0)] > 0)).astype(np.float32)

    # wrap gather indices per gather-call chunk of GPC groups (<=1024 idxs,
    # the HW per-dma_gather limit at 1KB rows) -> [16, n/16] tiled to 128 rows
    gpc = 8

    def wrap_idx(arr_a, cap):
        out = []
        ng = nblk * cap
        for c in range(ncores):
            rows = []
            for g0 in range(0, ng, gpc):
                gl = min(gpc, ng - g0)
                seg = arr_a[c, g0 * P : (g0 + gl) * P]
                w = seg.reshape(-1, 16).T  # [16, n/16]
                rows.append(np.tile(w, (8, 1)))
            out.append(np.concatenate(rows, axis=1))
        return np.stack(out).astype(np.int16)

    gidx_w = [wrap_idx(gidx[a], (cap_lo, cap_hi)[a]) for a in range(2)]
    # one-hot matrices, built host-side (pure index data):
    # oh[a][c][lane, group*P + d] = 1.0 iff edge at slot (group*P + lane) has
    # dst-lane d; pad slots (dloc=200) give all-zero rows.
    oh_w = []
    for a in range(2):
        per_core = []
        for c in range(ncores):
            dl = dloc[a][c].reshape(-1, P)            # [ngroups, lane] -> dstlane
            ng = dl.shape[0]
            ohm = np.zeros((ng, P, P), np.float32)    # [group, lane, d]
            gi, li = np.nonzero(dl < P)
            ohm[gi, li, dl[gi, li]] = 1.0
            # -> [lane(part), group*P + d]
            per_core.append(ohm.transpose(1, 0, 2).reshape(P, ng * P))
        oh_w.append(np.stack(per_core).astype(ml_dtypes.float8_e4m3))

    n8 = 0
    in_maps = []
    for c in range(ncores):
        in_maps.append({
            "h_lo": np.ascontiguousarray(h[:split]).astype(np.float32),
            "h_hi": np.ascontiguousarray(h[split:]).astype(np.float32),
            "h_ownt": h_ownT[c],
            "gidx_lo": gidx_w[0][c],
            "gidx_hi": gidx_w[1][c],
            "oh_lo": oh_w[0][c],
            "oh_hi": oh_w[1][c],
            "ct": ct[c].astype(np_bf16),
            "relt": np.ascontiguousarray(rel_emb.T).astype(np.float32),
            "w_n": weight_neighbor.astype(np.float32),
            "w_l": loop_weight.astype(np.float32),
            "w_e": evolve_loop_weight.astype(np.float32),
            "selm": np.tile(selm[c][None, :], (P, 1)).astype(np_bf16),
        })

    meta = dict(
        n_nodes=n_nodes, n_rel=n_rel, split=split, nblk=nblk, ls=ls,
        slots=slots, cap_lo=cap_lo, cap_hi=cap_hi, gpc=gpc, n8=n8,
        z_needed=bool(z_needed), node_of_slot=node_of_slot,
    )
    return in_maps, meta


# ----------------------------------------------------------------------------
# Device graph
# ----------------------------------------------------------------------------

def build_graph(meta, ncores=NCORES):
    FP8 = mybir.dt.float8e4
    n_nodes = meta["n_nodes"]
    n_rel = meta["n_rel"]
    split = meta["split"]
    nblk = meta["nblk"]
    ls = meta["ls"]
    slots = meta["slots"]
    cap_lo = meta["cap_lo"]
    cap_hi = meta["cap_hi"]
    gpc = meta["gpc"]
    n8 = meta["n8"]
    z_needed = meta["z_needed"]
    caps = (cap_lo, cap_hi)

    nc = bacc.Bacc("TRN2", target_bir_lowering=False, debug=False,
                   num_devices=ncores, num_swdge_queues=4)

    dt = nc.dram_tensor
    h_lo = dt("h_lo", [split, D], F32, kind="ExternalInput")
    h_hi = dt("h_hi", [n_nodes - split, D], F32, kind="ExternalInput")
    h_ownt = dt("h_ownt", [D, slots], F32, kind="ExternalInput")
    gidx = [dt("gidx_lo", [P, nblk * cap_lo * 8], I16, kind="ExternalInput"),
            dt("gidx_hi", [P, nblk * cap_hi * 8], I16, kind="ExternalInput")]
    FP8 = mybir.dt.float8e4
    ohd = [dt("oh_lo", [P, nblk * cap_lo * P], FP8, kind="ExternalInput"),
           dt("oh_hi", [P, nblk * cap_hi * P], FP8, kind="ExternalInput")]
    ct = dt("ct", [n_rel, slots], BF16, kind="ExternalInput")
    relt = dt("relt", [D, n_rel], F32, kind="ExternalInput")
    w_n = dt("w_n", [D, D], F32, kind="ExternalInput")
    w_l = dt("w_l", [D, D], F32, kind="ExternalInput")
    w_e = dt("w_e", [D, D], F32, kind="ExternalInput")
    selm = dt("selm", [P, ls * P], BF16, kind="ExternalInput")
    out = dt("out", [2, P, slots], BF16, kind="ExternalOutput")

    rel_k = [(k, min(P, n_rel - k)) for k in range(0, n_rel, P)]

    with tile.TileContext(nc) as tc:
        with (
            tc.tile_pool(name="dram", bufs=1, space="DRAM") as dp,
            tc.tile_pool(name="persist", bufs=1) as pp,
            tc.tile_pool(name="gather", bufs=3) as gp,
            tc.tile_pool(name="gbf", bufs=3) as gbp,
            tc.tile_pool(name="oh", bufs=4) as ohp,
            tc.tile_pool(name="wrk", bufs=2) as wp,
            tc.tile_pool(name="epi", bufs=3) as ep,
            tc.tile_pool(name="psum", bufs=8, space="PSUM") as psp,
        ):
            nc.gpsimd.load_library(library_config.mlp)
            h_src = [h_lo.ap(), h_hi.ap()]
            gidx_t = [pp.tile([P, nblk * caps[a] * 8], I16, tag=f"gidx{a}",
                              name=f"gidx_t{a}") for a in range(2)]
            for a in range(2):
                nc.sync.dma_start(gidx_t[a][:], gidx[a].ap())

            import os as _os
            _skip = set(_os.environ.get("KSKIP", "").split(","))

            # gather call k of arena a covers groups [k*gpc, k*gpc+gl);
            # rotate SWDGE queues: queue q runs on Q7 core pair {2q, 2q+1},
            # so consecutive calls on different queues overlap desc-gen.
            # SWDGE must carry ONLY these gathers so the round-robin DMASW
            # sem lanes each see a single queue.
            g_tiles = [{}, {}]
            qctr = [0]

            def ensure_call(a, call):
                if call in g_tiles[a]:
                    return
                cap = caps[a]
                ng = nblk * cap
                g0 = call * gpc
                gl = min(gpc, ng - g0)
                n_idx = gl * P
                gf = gp.tile([P, gpc, D], F32, tag=f"g{a}",
                             name=f"gf_{a}_{call}")
                nc.gpsimd.dma_gather(
                    gf[:, :gl, :], h_src[a],
                    gidx_t[a][:, g0 * 8:(g0 + gl) * 8],
                    n_idx, n_idx, D, queue_num=qctr[0] % 4,
                )
                qctr[0] += 1
                gb = gbp.tile([P, gpc, D], BF16, tag=f"gb{a}",
                              name=f"gb_{a}_{call}")
                nc.vector.tensor_copy(gb[:, :gl, :], gf[:, :gl, :])
                g_tiles[a][call] = gb

            # start the SWDGE/Q7 gather stream immediately; every other
            # load below overlaps behind it
            if "units" not in _skip:
                for _c in range(3):
                    ensure_call(0, _c)
                    ensure_call(1, _c)

            selm_t = pp.tile([P, ls * P], BF16, tag="selm")
            nc.sync.dma_start(selm_t[:], selm.ap())
            ct_t = [pp.tile([kn, slots], BF16, tag=f"ct{k}", name=f"ct_t{k}")
                    for k, kn in rel_k]
            for i, (k, kn) in enumerate(rel_k):
                nc.sync.dma_start(ct_t[i][:], ct.ap()[k:k + kn, :])

            # weights: load f32, cast to bf16 [K=D rows on partitions]
            def load_w_bf(w, tag):
                tiles = []
                for kc in range(D // P):
                    wf = wp.tile([P, D], F32, tag="wf32")
                    nc.sync.dma_start(wf[:], w.ap()[kc * P:(kc + 1) * P, :])
                    wb = pp.tile([P, D], BF16, tag=f"{tag}{kc}")
                    nc.vector.tensor_copy(wb[:], wf[:])
                    tiles.append(wb)
                return tiles

            w_n_t = load_w_bf(w_n, "wn")
            w_l_t = load_w_bf(w_l, "wl")
            w_e_t = load_w_bf(w_e, "we") if z_needed else None

            # rel_w = rel_emb @ Wn on-device from host-side rel^T layout
            relT = [pp.tile([P, n_rel], BF16, tag=f"relT{kc}",
                            name=f"relT{kc}") for kc in range(2)]
            for kc in range(2):
                rf = wp.tile([P, n_rel], F32, tag="relf32")
                nc.sync.dma_start(rf[:], relt.ap()[kc * P:(kc + 1) * P, :])
                nc.vector.tensor_copy(relT[kc][:], rf[:])
            relw_t = []
            for ki, (k, kn) in enumerate(rel_k):
                ps = psp.tile([P, D], F32, tag="ps")
                for kc in range(2):
                    nc.tensor.matmul(ps[:kn, :], relT[kc][:, k:k + kn],
                                     w_n_t[kc][:], start=(kc == 0),
                                     stop=(kc == 1))
                rw = pp.tile([kn, D], BF16, tag=f"relw{k}", name=f"relw{k}")
                nc.vector.tensor_copy(rw[:], ps[:kn, :])
                relw_t.append(rw)

            # ---- h_ownT: host ships [D, slots] f32; load + cast to bf16 ------
            ht = pp.tile([P, 2, slots], BF16, tag="ht")
            if "ht" in _skip:
                nc.vector.memset(ht[:], 0.0)
            for fh in ([] if "ht" in _skip else range(2)):
                for mb in range(0, nblk, 8):
                    w_ = min(8 * P, slots - mb * P)
                    hf = wp.tile([P, 8 * P], F32, tag="hf32")
                    nc.sync.dma_start(
                        hf[:, :w_],
                        h_ownt.ap()[fh * P:(fh + 1) * P, mb * P:mb * P + w_])
                    nc.scalar.activation(ht[:, fh, mb * P:mb * P + w_],
                                          hf[:, :w_],
                                          mybir.ActivationFunctionType.Identity)

            # ---- S^T accumulation: gather + one-hot matmuls ------------------
            st = pp.tile([P, 2, slots], BF16, tag="st")  # [feat-half][f, slot]
            if "units" in _skip:
                nc.vector.memset(st[:], 0.0)

            # ---- epilogue (emitted interleaved: tile t as soon as its 4 ------
            # blocks of st have been flushed, so it overlaps the gathers)
            ntiles = [(t * 512, 512) for t in range((nblk - ls) * P // 512)]
            ntiles += [((nblk - ls) * P + t * P, P) for t in range(ls)]
            sel_from = (nblk - ls) * P

            def emit_tile(n0, nw):
                for fh in range(2):
                    in_sel = z_needed and n0 >= sel_from
                    agg = psp.tile([P, nw], F32, tag="ps", name=f"agg{n0}_{fh}")
                    nmm = 2 + len(rel_k)
                    i = 0
                    for kc in range(2):
                        nc.tensor.matmul(
                            agg[:], w_n_t[kc][:, fh * P:(fh + 1) * P],
                            st[:, kc, n0:n0 + nw],
                            start=(i == 0), stop=(i == nmm - 1))
                        i += 1
                    for ki, (k, kn) in enumerate(rel_k):
                        nc.tensor.matmul(
                            agg[:], relw_t[ki][:, fh * P:(fh + 1) * P],
                            ct_t[ki][:, n0:n0 + nw],
                            start=(i == 0), stop=(i == nmm - 1))
                        i += 1
                    ot = ep.tile([P, nw], BF16, tag="ot", name=f"ot{n0}_{fh}")
                    nc.scalar.activation(ot[:], agg[:],
                                         mybir.ActivationFunctionType.Relu)
                    # reuse the agg bank for the loop-message chain (start=True
                    # resets it; Tile orders it after the relu's read)
                    lp = agg
                    for kc in range(2):
                        nc.tensor.matmul(
                            lp[:], w_l_t[kc][:, fh * P:(fh + 1) * P],
                            ht[:, kc, n0:n0 + nw],
                            start=(kc == 0), stop=(kc == 1))
                    if not in_sel:
                        nc.vector.tensor_add(ot[:], ot[:], lp[:])
                    else:
                        ep_ps = psp.tile([P, nw], F32, tag="ps",
                                         name=f"eps{n0}_{fh}")
                        for kc in range(2):
                            nc.tensor.matmul(
                                ep_ps[:], w_e_t[kc][:, fh * P:(fh + 1) * P],
                                ht[:, kc, n0:n0 + nw],
                                start=(kc == 0), stop=(kc == 1))
                        lsb = ep.tile([P, nw], F32, tag="lsb",
                                      name=f"lsb{n0}_{fh}")
                        m0 = n0 - sel_from
                        nc.vector.tensor_tensor(
                            lsb[:], lp[:], selm_t[:, m0:m0 + nw],
                            op=mybir.AluOpType.mult)
                        nc.vector.tensor_add(ot[:], ot[:], lsb[:])
                        # (1 - m) * E = E - m*E
                        me = ep.tile([P, nw], F32, tag="me",
                                     name=f"me{n0}_{fh}")
                        nc.vector.tensor_tensor(
                            me[:], ep_ps[:], selm_t[:, m0:m0 + nw],
                            op=mybir.AluOpType.mult)
                        nc.vector.tensor_tensor(
                            me[:], ep_ps[:], me[:],
                            op=mybir.AluOpType.subtract)
                        nc.vector.tensor_add(ot[:], ot[:], me[:])
                    nc.sync.dma_start(out.ap()[fh, :, n0:n0 + nw], ot[:])
            pending_tiles = list(ntiles)

            for b in ([] if "units" in _skip else range(nblk)):
                for a in range(2):
                    for g in range(caps[a]):
                        ensure_call(a, (b * caps[a] + g) // gpc)
                ohs = []
                for a in range(2):
                    oh8 = ohp.tile([P, caps[a] * P], FP8, tag=f"oh8{a}",
                                   name=f"oh8_{a}_{b}")
                    nc.sync.dma_start(
                        oh8[:],
                        ohd[a].ap()[:, b * caps[a] * P:(b + 1) * caps[a] * P])
                    oh = ohp.tile([P, caps[a] * P], BF16, tag=f"oh{a}",
                                  name=f"oh_{a}_{b}")
                    nc.scalar.activation(oh[:], oh8[:],
                                         mybir.ActivationFunctionType.Identity)
                    ohs.append(oh)
                for fh in range(2):
                    ps = psp.tile([P, P], F32, tag="ps", name=f"ps_{b}_{fh}")
                    tot = caps[0] + caps[1]
                    i = 0
                    for a in range(2):
                        for g in range(caps[a]):
                            gg = b * caps[a] + g
                            gb = g_tiles[a][gg // gpc]
                            nc.tensor.matmul(
                                ps[:],
                                gb[:, gg % gpc, fh * P:(fh + 1) * P],
                                ohs[a][:, g * P:(g + 1) * P],
                                start=(i == 0), stop=(i == tot - 1),
                            )
                            i += 1
                    nc.scalar.activation(
                        st[:, fh, b * P:(b + 1) * P], ps[:],
                        mybir.ActivationFunctionType.Identity)
                while (pending_tiles and
                       pending_tiles[0][0] + pending_tiles[0][1]
                       <= (b + 1) * P):
                    n0_, nw_ = pending_tiles.pop(0)
                    emit_tile(n0_, nw_)

            for n0_, nw_ in pending_tiles:
                emit_tile(n0_, nw_)


    nc.compile()
    return nc


# ----------------------------------------------------------------------------
# Entry point
# ----------------------------------------------------------------------------

_CACHE = {}


def _run(inputs, ncores=NCORES, sim=False, trace=False):
    h = np.asarray(inputs["h"], np.float32)
    rel_emb = np.asarray(inputs["rel_emb"], np.float32)
    w_n = np.asarray(inputs["weight_neighbor"], np.float32)
    w_l = np.asarray(inputs["loop_weight"], np.float32)
    w_e = np.asarray(inputs["evolve_loop_weight"], np.float32)
    src = np.asarray(inputs["src"])
    dst = np.asarray(inputs["dst"])
    etype = np.asarray(inputs["etype"])

    in_maps, meta = host_prep(h, rel_emb, w_n, w_l, w_e, src, dst, etype,
                              ncores=ncores)

    key = (meta["n_nodes"], meta["n_rel"], meta["split"], meta["nblk"],
           meta["cap_lo"], meta["cap_hi"], meta["ls"], meta["z_needed"], ncores)
    if key not in _CACHE:
        _CACHE[key] = build_graph(meta, ncores=ncores)
    nc = _CACHE[key]

    res = run_bass_kernel_spmd(nc, in_maps, core_ids=list(range(ncores)),
                               trace=trace)

    n_nodes = meta["n_nodes"]
    slots = meta["slots"]
    node_of_slot = meta["node_of_slot"]
    out_full = np.zeros((n_nodes, D), np.float32)
    for c in range(ncores):
        oc = np.asarray(res.results[c]["out"])  # [2, P, slots]
        oc = oc.reshape(2 * P, slots)  # [D(feature), slots]
        ns = node_of_slot[c]
        v = ns >= 0
        out_full[ns[v]] = oc[:, v].T
    return out_full, res, meta


def kernel(**inputs) -> np.ndarray:
    out, _, _ = _run(inputs)
    return out.astype(np.float32)


if __name__ == "__main__":
    # tiny smoke test with a synthetic small graph
    rng = np.random.default_rng(0)
    N_, E_, R_ = 2048, 16384, 16
    inputs = dict(
        h=rng.standard_normal((N_, D), dtype=np.float32),
        rel_emb=(rng.standard_normal((R_, D)) * 0.1).astype(np.float32),
        weight_neighbor=rng.standard_normal((D, D), dtype=np.float32) * 0.05,
        a=np.zeros((3 * D, 1), np.float32),
        W3=np.zeros((3 * D, 3 * D), np.float32),
        loop_weight=rng.standard_normal((D, D), dtype=np.float32) * 0.05,
        evolve_loop_weight=rng.standard_normal((D, D), dtype=np.float32) * 0.05,
        src=rng.integers(0, N_, E_),
        dst=rng.integers(0, N_, E_),
        etype=rng.integers(0, R_, E_),
    )
    out, res, meta = _run({k: v for k, v in inputs.items()
                           if k not in ("a", "W3")})
    # numpy reference
    S = np.zeros((N_, D), np.float32)
    np.add.at(S, inputs["dst"], inputs["h"][inputs["src"]]
              + inputs["rel_emb"][inputs["etype"]])
    indeg = np.bincount(inputs["dst"], minlength=N_)
    loopm = np.where((indeg > 0)[:, None],
                     inputs["h"] @ inputs["loop_weight"],
                     inputs["h"] @ inputs["evolve_loop_weight"])
    ref = np.maximum(S @ inputs["weight_neighbor"], 0) + loopm
    err = np.abs(out - ref).max() / (np.abs(ref).max() + 1e-9)
    print("small-graph rel err:", err)
